# revision 37
# baseline (speedup 1.0000x reference)
"""AttnBlock (GroupNorm + single-head self-attention + residual) on 8 TRN2 cores.

Sharding: data-parallel over (batch b, query-half h) -> 8 shards. Each core
receives the full [C, N] image of its batch (columns rolled so that its own
query half always occupies columns 0:NQ), computes GroupNorm stats + K/V over
the whole image, Q over its half, and a flash-style attention in which scores
are produced directly transposed (S^T = K^T.T @ Q^T tiles) so softmax
normalization needs no PE transposes of P.

The attention inner loop runs in fp8e4 with DoubleRow perf mode (2 contraction
rows per PE cell): one S matmul per key tile (contraction 256 = 2x128 channel
halves), PV over key-tile pairs, and the softmax denominator as a ones-vector
DoubleRow matmul accumulated into a [1, 512] PSUM row. exp() is applied to
key-tile PAIRS ([128, 1024] activations) to amortize ACT overhead, shifted by
-SHIFT so exp output fits fp8e4's +/-240 range (scores reach ~8).
Projections and the out-projection stay bf16 for accuracy.
"""

import os
import sys

import numpy as np

for _p in ("/opt/trn_rl_repo", "/root/.axon_site/_ro/trn_rl_repo"):
    if os.path.isdir(_p) and _p not in sys.path:
        sys.path.insert(0, _p)

import concourse.bass as bass  # noqa: E402
import concourse.tile as tile  # noqa: E402
from concourse import bacc, mybir  # noqa: E402
from concourse.masks import make_identity  # noqa: E402

# The agent image's antenv lacks axon_hooks; if BASS_TRACE is set in the
# environment, run_bass_kernel_spmd would crash importing it. Provide a stub
# (profiling degrades gracefully to "hook isn't registered").
try:
    import antenv.axon_hooks  # noqa: F401
except ImportError:
    import types as _types

    _m = _types.ModuleType("antenv.axon_hooks")
    _h = [None]
    _m.set_axon_ntff_profile_hook = lambda h: _h.__setitem__(0, h)
    _m.get_axon_ntff_profile_hook = lambda: _h[0]
    sys.modules["antenv.axon_hooks"] = _m

B, C, H, W = 4, 256, 64, 64
N = H * W  # 4096 pixels
NQ = N // 2  # 2048 queries per core
G = 32  # groups
CPG = C // G  # 8 channels per group
EPS = 1e-5
NCORES = 8
SCALE = float(C) ** -0.5  # 0.0625
SHIFT = 4.0  # exp(s - SHIFT): keeps exp <= ~50 << fp8e4 max 240

F32 = mybir.dt.float32
BF16 = mybir.dt.bfloat16
FP8 = mybir.dt.float8e4

QB = 512  # query block (free dim of S^T / PV matmuls)
NQB = NQ // QB  # 4 query blocks
NKT = N // 128  # 32 key tiles
NKP = NKT // 2  # 16 key-tile pairs
NNB = N // QB  # 8 pixel blocks for K/V projections
P = 128

Act = mybir.ActivationFunctionType
Alu = mybir.AluOpType
Axis = mybir.AxisListType
DR = mybir.MatmulPerfMode.DoubleRow

_NC = None
LAST_RESULTS = None


def _body(tc, d):
    nc = tc.nc
    x_d = d["x"]
    out_d = d["out"]

    const = tc.alloc_tile_pool(name="const", bufs=1)
    small = tc.alloc_tile_pool(name="small", bufs=1)
    pblk = tc.alloc_tile_pool(name="pblk", bufs=2)
    work = tc.alloc_tile_pool(name="work", bufs=2)
    # PSUM budget (8 banks): "sps" 4KB x2 = 4, "po" 2KB x1 = 1, acc 2, dps 1
    ps = tc.alloc_tile_pool(name="ps", bufs=2, space="PSUM")
    ps_acc = tc.alloc_tile_pool(name="ps_acc", bufs=2, space="PSUM")
    ps_d = tc.alloc_tile_pool(name="ps_d", bufs=1, space="PSUM")

    # ---- constants issued first so every engine's stream opens with
    # dependency-free work (PE warm-up, ACT table preload) ----
    wu_w = const.tile([P, P], BF16)
    nc.vector.memset(wu_w, 0.0)
    wu_x = const.tile([P, QB], BF16)
    nc.vector.memset(wu_x, 0.0)
    wu_ps = ps.tile([P, QB], F32, name="wu_ps", tag="po", bufs=1)

    def warm(n):
        for _ in range(n):
            nc.tensor.matmul(wu_ps, lhsT=wu_w, rhs=wu_x, start=True, stop=True)

    ident = const.tile([P, P], F32)
    make_identity(nc, ident)
    one11 = const.tile([1, 1], F32)
    nc.vector.memset(one11, 1.0)
    ones_f = const.tile([P, 1], F32)
    nc.vector.memset(ones_f, 1.0)
    # fp8 "ones" pair for the denominator DoubleRow matmul. Padded free dim so
    # the pair-dim byte step is 16 (ISA requires step % 16 == 0).
    ones8 = const.tile([P, 2, 16], FP8)
    nc.vector.memset(ones8, 1.0)
    eps11 = const.tile([1, 1], F32)
    nc.vector.memset(eps11, EPS)
    shift_col = const.tile([P, 1], F32)
    nc.vector.memset(shift_col, -SHIFT)

    # ---- bulk DMAs first, on the two hardware DGE queues (sync + scalar).
    # Weights are host-packed into one [128, 2048] f32 tensor (8KB contiguous
    # per partition line); x arrives twice from host: bf16 (stats + residual)
    # and fp8 (projection matmul operand). x halves use 4KB lines so bn_stats
    # can start on the first half while the rest streams in. ----
    x_sb = const.tile([P, 2, N], BF16)
    x_bf = x_sb
    x8_sb = const.tile([P, 2, N], FP8)
    wall_sb = const.tile([P, 4, 2, C], F32)
    x_src = x_d.ap().rearrange("(h p) n -> p h n", p=P)
    x8_src = d["x8"].ap().rearrange("(h p) n -> p h n", p=P)
    for half in range(2):
        for ch in range(2):
            sl = (slice(None), ch, slice(half * 4 * QB, (half + 1) * 4 * QB))
            eng = nc.sync if ch == 0 else nc.scalar
            eng.dma_start(out=x_sb[sl], in_=x_src[sl])
    nc.scalar.dma_start(
        out=wall_sb.rearrange("p k h c -> p (k h c)"), in_=d["wall"].ap()
    )
    for ch in range(2):
        nc.sync.dma_start(
            out=x8_sb[:, ch, :], in_=x8_src[:, ch, :]
        )

    bn_st = [small.tile([P, NNB, 6], F32, name=f"bnst_{ch}") for ch in range(2)]
    for half in range(2):
        for ch in range(2):
            for j in range(4):
                jj = half * 4 + j
                nc.vector.bn_stats(
                    out=bn_st[ch][:, jj, :],
                    in_=x_sb[:, ch, jj * QB:(jj + 1) * QB],
                )

    # PE warm-up: keep the HAM activity monitor busy during the DMA/stats
    # window so projections and attention run at full clock from the start.
    warm(56)

    # preload the ACT exp and sqrt tables before the dance/attention need them
    # (issued after the scalar-queue DMAs so they don't delay the transfers)
    warm11 = small.tile([1, 1], F32)
    nc.scalar.activation(warm11, one11, Act.Exp, scale=1.0)
    warm12 = small.tile([1, 1], F32)
    nc.scalar.activation(warm12, one11, Act.Sqrt, scale=1.0)

    gam_row = const.tile([1, C], F32)
    nc.gpsimd.dma_start(out=gam_row, in_=d["gamma"][:, :])
    bet_row = const.tile([1, C], F32)
    nc.gpsimd.dma_start(out=bet_row, in_=d["beta"][:, :])

    # per-partition bias columns [128,1] x 2 channel-halves
    bias_cols = {}
    for nm in ("bq", "bk", "bv", "bo"):
        cols = []
        for ch in range(2):
            t = const.tile([P, 1], F32, name=f"{nm}_{ch}")
            nc.gpsimd.dma_start(out=t, in_=d[nm][ch * P:(ch + 1) * P, :])
            cols.append(t)
        bias_cols[nm] = cols

    # ---- GroupNorm statistics (bn_stats already issued in the DMA loop) ----
    mv = []
    for ch in range(2):
        m = small.tile([P, 2], F32, name=f"mv_{ch}")
        nc.vector.bn_aggr(out=m, in_=bn_st[ch])
        mv.append(m)

    # transpose per-channel mean and var into one row [1, 512]:
    # [mean_c0 | mean_c1 | var_c0 | var_c1]
    tp = ps.tile([1, 4 * P], F32, name="tp_stat", tag="po", bufs=1)
    for ch in range(2):
        for k in range(2):
            nc.tensor.transpose(
                tp[:, (2 * k + ch) * P:(2 * k + ch + 1) * P], mv[ch][:, k:k + 1],
                ident,
            )
    mrows = small.tile([1, 4 * P], F32)
    nc.vector.tensor_copy(out=mrows, in_=tp)
    mean_row = mrows[:, 0:C]
    var_row = mrows[:, C:2 * C]

    warm(8)  # keep the PE activity monitor warm through the stats dance

    # group stats (sums over the 8 channels of each group)
    msq_row = small.tile([1, C], F32)
    nc.vector.tensor_mul(msq_row, mean_row, mean_row)
    ex2_row = small.tile([1, C], F32)
    nc.vector.tensor_add(ex2_row, msq_row, var_row)
    m_s = small.tile([1, G], F32)
    nc.vector.tensor_reduce(
        out=m_s, in_=mean_row.rearrange("o (g j) -> o g j", j=CPG), axis=Axis.X,
        op=Alu.add,
    )
    e_s = small.tile([1, G], F32)
    nc.vector.tensor_reduce(
        out=e_s, in_=ex2_row.rearrange("o (g j) -> o g j", j=CPG), axis=Axis.X,
        op=Alu.add,
    )
    # var_g = e_s/8 - (m_s/8)^2
    mm_g = small.tile([1, G], F32)
    nc.vector.tensor_mul(mm_g, m_s, m_s)
    mm_g2 = small.tile([1, G], F32)
    nc.vector.tensor_scalar_mul(mm_g2, mm_g, 1.0 / (CPG * CPG))
    var_g = small.tile([1, G], F32)
    nc.vector.scalar_tensor_tensor(
        out=var_g, in0=e_s, scalar=1.0 / CPG, in1=mm_g2, op0=Alu.mult,
        op1=Alu.subtract,
    )
    # rstd_g = 1/sqrt(var_g + eps). The sqrt is issued BEFORE the w_bf casts
    # so the dance's one ACT op isn't queued behind four big weight casts.
    sq_g = small.tile([1, G], F32)
    nc.scalar.activation(sq_g, var_g, Act.Sqrt, bias=eps11, scale=1.0)
    rstd_g = small.tile([1, G], F32)
    nc.vector.reciprocal(rstd_g, sq_g)

    # ---- weights: cast to bf16 [128(ci), 2(ci_half), 256(co)]; wot also to
    # fp8 x8 for the DoubleRow out-projection ----
    w_bf = {}
    for wi, nm in enumerate(("wqt", "wkt", "wvt", "wot")):
        wb = const.tile([P, 2, C], BF16, name=f"{nm}_bf")
        for ch in range(2):
            nc.scalar.copy(wb[:, ch, :], wall_sb[:, wi, ch, :])
        w_bf[nm] = wb
    wot8 = const.tile([P, 2, C], FP8)
    for ch in range(2):
        nc.scalar.mul(wot8[:, ch, :], wall_sb[:, 3, ch, :], 8.0)

    # broadcast group values to channels: [1,32] -> [1,256] (repeat 8) in one
    # DVE copy via a step-0 read AP
    def grp_bcast(src, name):
        dst = small.tile([1, C], F32, name=name)
        src_ap = src[:, :]
        rep = bass.AP(
            tensor=src_ap.tensor, offset=src_ap.offset,
            ap=[src_ap.ap[0], src_ap.ap[1], [0, CPG]],
        )
        nc.vector.tensor_copy(out=dst.rearrange("o (g j) -> o g j", j=CPG), in_=rep)
        return dst

    rstd_c = grp_bcast(rstd_g, "rstd_c")
    msum_c = grp_bcast(m_s, "msum_c")

    # a = gamma * rstd ; b = beta - (m_s/8) * a    (rows [1,256])
    a_row = small.tile([1, C], F32)
    nc.vector.tensor_mul(a_row, gam_row, rstd_c)
    ma_row = small.tile([1, C], F32)
    nc.vector.scalar_tensor_tensor(
        out=ma_row, in0=msum_c, scalar=1.0 / CPG, in1=a_row, op0=Alu.mult,
        op1=Alu.mult,
    )
    b_row = small.tile([1, C], F32)
    nc.vector.tensor_sub(b_row, bet_row, ma_row)

    # transpose a/b rows back to per-partition columns [128,1] per ch-half
    ab_cols = {"a": [], "b": []}
    for nm, row in (("a", a_row), ("b", b_row)):
        for ch in range(2):
            tp = ps.tile([P, 1], F32, name="tp_ab", tag="po", bufs=1)
            nc.tensor.matmul(
                tp, lhsT=row[:, ch * P:(ch + 1) * P], rhs=one11, start=True,
                stop=True,
            )
            col = small.tile([P, 1], F32, name=f"{nm}_col_{ch}")
            nc.vector.tensor_copy(out=col, in_=tp)
            ab_cols[nm].append(col)

    # ---- fold the norm affine into the projections ----
    # Q = (wq diag(a)) x_bf + (wq b + bq), same for K; V likewise with its
    # constant (wv b + bv) folded through PV/denom into bo_eff.
    b_bf = []
    for ci in range(2):
        t = small.tile([P, 1], BF16, name=f"b_bf_{ci}")
        nc.vector.tensor_copy(out=t, in_=ab_cols["b"][ci])
        b_bf.append(t)

    def matvec_bias(wname, rhs_cols, bias_add, out_dt, out_name):
        outs = []
        for co in range(2):
            pe = ps.tile([P, 1], F32, name="pe_mv", tag="po", bufs=1)
            for ci in range(2):
                nc.tensor.matmul(
                    pe, lhsT=w_bf[wname][:, ci, co * P:(co + 1) * P],
                    rhs=rhs_cols[ci], start=(ci == 0), stop=(ci == 1),
                )
            t = small.tile([P, 1], out_dt, name=f"{out_name}_{co}")
            nc.scalar.activation(
                t, pe, Act.Identity, bias=bias_add[co], scale=1.0
            )
            outs.append(t)
        return outs

    be_q = matvec_bias("wqt", b_bf, bias_cols["bq"], F32, "be_q")
    be_k = matvec_bias("wkt", b_bf, bias_cols["bk"], F32, "be_k")
    vbv_bf = matvec_bias("wvt", b_bf, bias_cols["bv"], BF16, "vbv")
    bo_eff = matvec_bias("wot", vbv_bf, bias_cols["bo"], F32, "bo_eff")
    warm(8)  # cover the be/w8 latency gap before the projections

    # scale wq/wk/wv rows by 8*a (per input channel) into fp8 tiles for the
    # DoubleRow projection matmuls; the 8x (for fp8 dynamic range on the
    # small weight values) is undone by the 1/8 in the PSUM->fp8 casts.
    a8_cols = []
    for ci in range(2):
        t = small.tile([P, 1], F32, name=f"a8_{ci}")
        nc.vector.tensor_scalar_mul(t, ab_cols["a"][ci], 8.0)
        a8_cols.append(t)
    w8 = {}
    for wname in ("wqt", "wkt", "wvt"):
        ws = const.tile([P, 2, C], FP8, name=f"{wname}_8")
        for ci in range(2):
            nc.vector.tensor_scalar_mul(
                ws[:, ci, :], w_bf[wname][:, ci, :], a8_cols[ci]
            )
        w8[wname] = ws

    # ---- projections: fp8 DoubleRow matmuls (contraction 256 = 2 ci halves
    # per instruction) over host-supplied x8, PSUM casts apply the 1/8 that
    # undoes the 8x in w8. Q/K in nb-PAIR psum tiles (4KB) so one [P, 1024]
    # cast amortizes ACT/DVE per-op overhead. ----
    k_sb = const.tile([P, 2, N], FP8)
    q_sb = const.tile([P, 2, NQ], FP8)

    def proj_pair(wname, dst, be, nbp, co):
        pp = ps.tile([P, 2, QB], F32, name="pp", tag="sps")
        for j in range(2):
            nb = 2 * nbp + j
            nc.tensor.matmul(
                pp[:, j, :], lhsT=w8[wname][:, :, co * P:(co + 1) * P],
                rhs=x8_sb[:, :, nb * QB:(nb + 1) * QB],
                start=True, stop=True, perf_mode=DR,
            )
        # split the [P, 1024] cast across ACT and DVE so neither engine's
        # latency gates the (PE-bound) projection pipeline
        for j in range(2):
            dcols = dst[:, co, (nbp * 2 + j) * QB:(nbp * 2 + j + 1) * QB]
            if j == 0:
                nc.scalar.activation(
                    dcols, pp[:, j, :], Act.Identity, bias=be[co], scale=0.125,
                )
            else:
                nc.vector.tensor_scalar(
                    out=dcols, in0=pp[:, j, :],
                    scalar1=0.125, scalar2=be[co], op0=Alu.mult, op1=Alu.add,
                )

    for co in range(2):
        proj_pair("wqt", q_sb, be_q, 0, co)
    for nbp in range(4):
        for co in range(2):
            proj_pair("wkt", k_sb, be_k, nbp, co)
    for co in range(2):
        proj_pair("wqt", q_sb, be_q, 1, co)

    # V [N, C] fp8 (bias folded into bo_eff) in 4-chunk psum tiles:
    # psum[:, n2*C:+C] = sum_ci x8_chunk.T @ w8v
    v_sb = const.tile([P, NKT, C], FP8)
    v_flat = v_sb.rearrange("p k c -> p (k c)")
    for nt in range(0, NKT, 4):
        pv = ps.tile([P, 4 * C], F32, name="pv", tag="sps")
        for n2 in range(4):
            nc.tensor.matmul(
                pv[:, n2 * C:(n2 + 1) * C],
                lhsT=x8_sb[:, :, (nt + n2) * P:(nt + n2 + 1) * P],
                rhs=w8["wvt"][:, :, :],
                start=True, stop=True, perf_mode=DR,
            )
        nc.scalar.mul(v_flat[:, nt * C:(nt + 2) * C], pv[:, 0:2 * C], 0.125)
        nc.vector.tensor_scalar(
            out=v_flat[:, (nt + 2) * C:(nt + 4) * C], in0=pv[:, 2 * C:4 * C],
            scalar1=0.125, scalar2=None, op0=Alu.mult,
        )

    # ---- attention, per query block; key tiles processed in PAIRS with fp8
    # DoubleRow matmuls (contraction 256 per instruction). The softmax
    # denominator accumulates on the PE as a ones-vector DoubleRow matmul
    # into dps [1, 512]. The division is commuted through the out-projection:
    # out = (wo @ (P.V)) * (1/denom) + bo_eff + x.
    def epilogue(qb, dps, aps, last=False):
        # casts first (both DVE; scale 1/8 for the fp8 out-projection): they
        # release the PV accumulator banks immediately
        at8 = work.tile([P, 2, QB], FP8, name="at8", tag="at8", bufs=2)
        for ci in range(2):
            nc.vector.tensor_scalar(
                out=at8[:, ci, :], in0=aps[ci],
                scalar1=0.125, scalar2=None, op0=Alu.mult,
            )
        den_r = work.tile([1, QB], F32, name="den_r", tag="den_r")
        nc.vector.reciprocal_approx_fast(out=den_r, in_=dps)
        den_b = work.tile([P, QB], F32, name="den_b", tag="den_b", bufs=2)
        nc.gpsimd.partition_broadcast(den_b, den_r)
        for co in range(2):
            po = ps.tile([P, QB], F32, name="po", tag="po", bufs=1)
            nc.tensor.matmul(
                po, lhsT=wot8[:, :, co * P:(co + 1) * P],
                rhs=at8[:, :, :], start=True, stop=True, perf_mode=DR,
            )
            t1 = work.tile([P, QB], F32, name="t1", tag="t1")
            nc.vector.tensor_mul(t1, po, den_b)
            res = work.tile([P, QB], BF16, name="res", tag="res", bufs=4)
            nc.vector.scalar_tensor_tensor(
                out=res, in0=t1, scalar=bo_eff[co],
                in1=x_sb[:, co, qb * QB:(qb + 1) * QB], op0=Alu.add, op1=Alu.add,
            )
            eng = nc.sync if co == 0 else nc.scalar
            eng.dma_start(
                out=out_d[co * P:(co + 1) * P, qb * QB:(qb + 1) * QB], in_=res
            )

    pending = None
    for qb in range(NQB):
        p_sb = pblk.tile([P, NKT, QB], FP8, name="p_sb")
        dps = ps_d.tile([1, QB], F32, name="dps")
        aps = [
            ps_acc.tile([P, QB], F32, name="aps", tag="acc") for _ in range(2)
        ]
        for kp in range(NKP + 2):
            if kp == 1 and pending is not None:
                # previous qb's epilogue goes FIRST so its at_sb casts
                # precede this qb's exps in the ACT/DVE program order
                epilogue(*pending)
                pending = None
            if kp < NKP:
                sps2 = ps.tile([P, 2, QB], F32, name="sps2", tag="sps")
                for j in range(2):
                    kt = 2 * kp + j
                    nc.tensor.matmul(
                        sps2[:, j, :],
                        lhsT=k_sb[:, :, kt * P:(kt + 1) * P],
                        rhs=q_sb[:, :, qb * QB:(qb + 1) * QB],
                        start=True, stop=True, perf_mode=DR,
                    )
                nc.scalar.activation(
                    p_sb[:, 2 * kp:2 * kp + 2, :].rearrange("p k q -> p (k q)"),
                    sps2.rearrange("p k q -> p (k q)"),
                    Act.Exp, scale=SCALE, bias=shift_col,
                )
            if kp >= 2:
                pk = kp - 2
                nc.tensor.matmul(
                    dps, lhsT=ones8[:, :, 0:1],
                    rhs=p_sb[:, 2 * pk:2 * pk + 2, :],
                    start=(pk == 0), stop=(pk == NKP - 1),
                    perf_mode=DR, skip_group_check=True,
                )
                for ch in range(2):
                    nc.tensor.matmul(
                        aps[ch],
                        lhsT=v_sb[:, 2 * pk:2 * pk + 2, ch * P:(ch + 1) * P],
                        rhs=p_sb[:, 2 * pk:2 * pk + 2, :],
                        start=(pk == 0), stop=(pk == NKP - 1),
                        perf_mode=DR, skip_group_check=True,
                    )
        pending = (qb, dps, aps)
    epilogue(*pending, last=True)

    for pool in (ps_d, ps_acc, ps, work, pblk, small, const):
        pool.release()


def build_program():
    global _NC
    if _NC is not None:
        return _NC
    nc = bacc.Bacc("TRN2", target_bir_lowering=False, debug=False,
                   num_devices=NCORES)
    d = {
        "x": nc.dram_tensor("x", [C, N], BF16, kind="ExternalInput"),
        "x8": nc.dram_tensor("x8", [C, N], FP8, kind="ExternalInput"),
        "wall": nc.dram_tensor("wall", [P, 4 * 2 * C], F32, kind="ExternalInput"),
        "bq": nc.dram_tensor("bq", [C, 1], F32, kind="ExternalInput"),
        "bk": nc.dram_tensor("bk", [C, 1], F32, kind="ExternalInput"),
        "bv": nc.dram_tensor("bv", [C, 1], F32, kind="ExternalInput"),
        "bo": nc.dram_tensor("bo", [C, 1], F32, kind="ExternalInput"),
        "gamma": nc.dram_tensor("gamma", [1, C], F32, kind="ExternalInput"),
        "beta": nc.dram_tensor("beta", [1, C], F32, kind="ExternalInput"),
        "out": nc.dram_tensor("out", [C, NQ], BF16, kind="ExternalOutput"),
    }
    with tile.TileContext(nc) as tc:
        _body(tc, d)
    nc.compile()
    _NC = nc
    return nc


def make_in_maps(x, gamma, beta, wq, bq, wk, bk, wv, bv, wo, bo):
    f32c = lambda a: np.ascontiguousarray(np.asarray(a, dtype=np.float32))
    x = f32c(x)
    # wall[p, k, h, co] = w_k^T[h*128+p, co]  (k in {q,k,v,o})
    wall = np.stack([
        np.asarray(w, np.float32).T for w in (wq, wk, wv, wo)
    ]).reshape(4, 2, P, C).transpose(2, 0, 1, 3).reshape(P, 4 * 2 * C)
    base = {
        "wall": f32c(wall),
        "bq": f32c(bq).reshape(C, 1),
        "bk": f32c(bk).reshape(C, 1),
        "bv": f32c(bv).reshape(C, 1),
        "bo": f32c(bo).reshape(C, 1),
        "gamma": f32c(gamma).reshape(1, C),
        "beta": f32c(beta).reshape(1, C),
    }
    import ml_dtypes

    in_maps = []
    for core in range(NCORES):
        b, h = divmod(core, 2)
        xb = x[b].reshape(C, N)
        if h:
            xb = np.concatenate([xb[:, NQ:], xb[:, :NQ]], axis=1)
        in_maps.append({
            **base,
            "x": np.ascontiguousarray(xb.astype(ml_dtypes.bfloat16)),
            "x8": np.ascontiguousarray(xb.astype(ml_dtypes.float8_e4m3)),
        })
    return in_maps


def kernel(x, gamma, beta, wq, bq, wk, bk, wv, bv, wo, bo):
    global LAST_RESULTS
    from concourse.bass_utils import run_bass_kernel_spmd

    nc = build_program()
    in_maps = make_in_maps(x, gamma, beta, wq, bq, wk, bk, wv, bv, wo, bo)
    res = run_bass_kernel_spmd(nc, in_maps, core_ids=list(range(NCORES)))
    LAST_RESULTS = res
    out = np.empty((B, C, N), np.float32)
    for core in range(NCORES):
        b, h = divmod(core, 2)
        out[b][:, h * NQ:(h + 1) * NQ] = np.asarray(
            res.results[core]["out"], dtype=np.float32
        )
    return out.reshape(B, C, H, W)


# revision 40
# speedup vs baseline: 1.0104x; 1.0104x over previous
"""AttnBlock (GroupNorm + single-head self-attention + residual) on 8 TRN2 cores.

Sharding: data-parallel over (batch b, query-half h) -> 8 shards. Each core
receives the full [C, N] image of its batch (columns rolled so that its own
query half always occupies columns 0:NQ), computes GroupNorm stats + K/V over
the whole image, Q over its half, and a flash-style attention in which scores
are produced directly transposed (S^T = K^T.T @ Q^T tiles) so softmax
normalization needs no PE transposes of P.

The attention inner loop runs in fp8e4 with DoubleRow perf mode (2 contraction
rows per PE cell): one S matmul per key tile (contraction 256 = 2x128 channel
halves), PV over key-tile pairs, and the softmax denominator as a ones-vector
DoubleRow matmul accumulated into a [1, 512] PSUM row. exp() is applied to
key-tile PAIRS ([128, 1024] activations) to amortize ACT overhead, shifted by
-SHIFT so exp output fits fp8e4's +/-240 range (scores reach ~8).
Projections and the out-projection stay bf16 for accuracy.
"""

import os
import sys

import numpy as np

for _p in ("/opt/trn_rl_repo", "/root/.axon_site/_ro/trn_rl_repo"):
    if os.path.isdir(_p) and _p not in sys.path:
        sys.path.insert(0, _p)

import concourse.bass as bass  # noqa: E402
import concourse.tile as tile  # noqa: E402
from concourse import bacc, mybir  # noqa: E402
from concourse.masks import make_identity  # noqa: E402

# The agent image's antenv lacks axon_hooks; if BASS_TRACE is set in the
# environment, run_bass_kernel_spmd would crash importing it. Provide a stub
# (profiling degrades gracefully to "hook isn't registered").
try:
    import antenv.axon_hooks  # noqa: F401
except ImportError:
    import types as _types

    _m = _types.ModuleType("antenv.axon_hooks")
    _h = [None]
    _m.set_axon_ntff_profile_hook = lambda h: _h.__setitem__(0, h)
    _m.get_axon_ntff_profile_hook = lambda: _h[0]
    sys.modules["antenv.axon_hooks"] = _m

B, C, H, W = 4, 256, 64, 64
N = H * W  # 4096 pixels
NQ = N // 2  # 2048 queries per core
G = 32  # groups
CPG = C // G  # 8 channels per group
EPS = 1e-5
NCORES = 8
SCALE = float(C) ** -0.5  # 0.0625
SHIFT = 4.0  # exp(s - SHIFT): keeps exp <= ~50 << fp8e4 max 240

F32 = mybir.dt.float32
BF16 = mybir.dt.bfloat16
FP8 = mybir.dt.float8e4

QB = 512  # query block (free dim of S^T / PV matmuls)
NQB = NQ // QB  # 4 query blocks
NKT = N // 128  # 32 key tiles
NKP = NKT // 2  # 16 key-tile pairs
NNB = N // QB  # 8 pixel blocks for K/V projections
P = 128

Act = mybir.ActivationFunctionType
Alu = mybir.AluOpType
Axis = mybir.AxisListType
DR = mybir.MatmulPerfMode.DoubleRow

_NC = None
LAST_RESULTS = None


def _body(tc, d):
    nc = tc.nc
    x_d = d["x"]
    out_d = d["out"]

    const = tc.alloc_tile_pool(name="const", bufs=1)
    small = tc.alloc_tile_pool(name="small", bufs=1)
    pblk = tc.alloc_tile_pool(name="pblk", bufs=2)
    work = tc.alloc_tile_pool(name="work", bufs=2)
    # PSUM budget (8 banks): "sps" 4KB x2 = 4, "po" 2KB x1 = 1, acc 2, dps 1
    ps = tc.alloc_tile_pool(name="ps", bufs=2, space="PSUM")
    ps_acc = tc.alloc_tile_pool(name="ps_acc", bufs=2, space="PSUM")
    ps_d = tc.alloc_tile_pool(name="ps_d", bufs=1, space="PSUM")

    # ---- constants issued first so every engine's stream opens with
    # dependency-free work (PE warm-up, ACT table preload) ----
    wu_w = const.tile([P, P], BF16)
    nc.vector.memset(wu_w, 0.0)
    wu_x = const.tile([P, QB], BF16)
    nc.vector.memset(wu_x, 0.0)
    wu_ps = ps.tile([P, QB], F32, name="wu_ps", tag="po", bufs=1)

    def warm(n, cols=QB):
        for _ in range(n):
            nc.tensor.matmul(
                wu_ps[:, 0:cols], lhsT=wu_w, rhs=wu_x[:, 0:cols],
                start=True, stop=True,
            )

    ident = const.tile([P, P], F32)
    make_identity(nc, ident)
    one11 = const.tile([1, 1], F32)
    nc.vector.memset(one11, 1.0)
    ones_f = const.tile([P, 1], F32)
    nc.vector.memset(ones_f, 1.0)
    # fp8 "ones" pair for the denominator DoubleRow matmul. Padded free dim so
    # the pair-dim byte step is 16 (ISA requires step % 16 == 0).
    ones8 = const.tile([P, 2, 16], FP8)
    nc.vector.memset(ones8, 1.0)
    eps11 = const.tile([1, 1], F32)
    nc.vector.memset(eps11, EPS)
    shift_col = const.tile([P, 1], F32)
    nc.vector.memset(shift_col, -SHIFT)

    # ---- bulk DMAs first, on the two hardware DGE queues (sync + scalar).
    # Weights are host-packed into one [128, 2048] f32 tensor (8KB contiguous
    # per partition line); x arrives twice from host: bf16 (stats + residual)
    # and fp8 (projection matmul operand). x halves use 4KB lines so bn_stats
    # can start on the first half while the rest streams in. ----
    x_sb = const.tile([P, 2, N], BF16)
    x_bf = x_sb
    x8_sb = const.tile([P, 2, N], FP8)
    wall_sb = const.tile([P, 4, 2, C], F32)
    x_src = x_d.ap().rearrange("(h p) n -> p h n", p=P)
    x8_src = d["x8"].ap().rearrange("(h p) n -> p h n", p=P)
    for half in range(2):
        for ch in range(2):
            sl = (slice(None), ch, slice(half * 4 * QB, (half + 1) * 4 * QB))
            eng = nc.sync if ch == 0 else nc.scalar
            eng.dma_start(out=x_sb[sl], in_=x_src[sl])
    nc.scalar.dma_start(
        out=wall_sb.rearrange("p k h c -> p (k h c)"), in_=d["wall"].ap()
    )
    for ch in range(2):
        nc.sync.dma_start(
            out=x8_sb[:, ch, :], in_=x8_src[:, ch, :]
        )

    bn_st = [small.tile([P, NNB, 6], F32, name=f"bnst_{ch}") for ch in range(2)]
    for half in range(2):
        for ch in range(2):
            for j in range(4):
                jj = half * 4 + j
                nc.vector.bn_stats(
                    out=bn_st[ch][:, jj, :],
                    in_=x_sb[:, ch, jj * QB:(jj + 1) * QB],
                )

    # PE warm-up: keep the HAM activity monitor busy during the DMA/stats
    # window so projections and attention run at full clock from the start.
    warm(56)

    # preload the ACT exp and sqrt tables before the dance/attention need them
    # (issued after the scalar-queue DMAs so they don't delay the transfers)
    warm11 = small.tile([1, 1], F32)
    nc.scalar.activation(warm11, one11, Act.Exp, scale=1.0)
    warm12 = small.tile([1, 1], F32)
    nc.scalar.activation(warm12, one11, Act.Sqrt, scale=1.0)

    gam_row = const.tile([1, C], F32)
    nc.gpsimd.dma_start(out=gam_row, in_=d["gamma"][:, :])
    bet_row = const.tile([1, C], F32)
    nc.gpsimd.dma_start(out=bet_row, in_=d["beta"][:, :])

    # per-partition bias columns [128,1] x 2 channel-halves
    bias_cols = {}
    for nm in ("bq", "bk", "bv", "bo"):
        cols = []
        for ch in range(2):
            t = const.tile([P, 1], F32, name=f"{nm}_{ch}")
            nc.gpsimd.dma_start(out=t, in_=d[nm][ch * P:(ch + 1) * P, :])
            cols.append(t)
        bias_cols[nm] = cols

    # ---- GroupNorm statistics (bn_stats already issued in the DMA loop) ----
    mv = []
    for ch in range(2):
        m = small.tile([P, 2], F32, name=f"mv_{ch}")
        nc.vector.bn_aggr(out=m, in_=bn_st[ch])
        mv.append(m)

    # transpose per-channel mean and var into one row [1, 512]:
    # [mean_c0 | mean_c1 | var_c0 | var_c1]
    tp = ps.tile([1, 4 * P], F32, name="tp_stat", tag="po", bufs=1)
    for ch in range(2):
        for k in range(2):
            nc.tensor.transpose(
                tp[:, (2 * k + ch) * P:(2 * k + ch + 1) * P], mv[ch][:, k:k + 1],
                ident,
            )
    mrows = small.tile([1, 4 * P], F32)
    nc.vector.tensor_copy(out=mrows, in_=tp)
    mean_row = mrows[:, 0:C]
    var_row = mrows[:, C:2 * C]

    warm(8)  # keep the PE activity monitor warm through the stats dance

    # group stats (sums over the 8 channels of each group)
    msq_row = small.tile([1, C], F32)
    nc.vector.tensor_mul(msq_row, mean_row, mean_row)
    ex2_row = small.tile([1, C], F32)
    nc.vector.tensor_add(ex2_row, msq_row, var_row)
    m_s = small.tile([1, G], F32)
    nc.vector.tensor_reduce(
        out=m_s, in_=mean_row.rearrange("o (g j) -> o g j", j=CPG), axis=Axis.X,
        op=Alu.add,
    )
    e_s = small.tile([1, G], F32)
    nc.vector.tensor_reduce(
        out=e_s, in_=ex2_row.rearrange("o (g j) -> o g j", j=CPG), axis=Axis.X,
        op=Alu.add,
    )
    # var_g = e_s/8 - (m_s/8)^2
    mm_g = small.tile([1, G], F32)
    nc.vector.tensor_mul(mm_g, m_s, m_s)
    mm_g2 = small.tile([1, G], F32)
    nc.vector.tensor_scalar_mul(mm_g2, mm_g, 1.0 / (CPG * CPG))
    var_g = small.tile([1, G], F32)
    nc.vector.scalar_tensor_tensor(
        out=var_g, in0=e_s, scalar=1.0 / CPG, in1=mm_g2, op0=Alu.mult,
        op1=Alu.subtract,
    )
    # rstd_g = 1/sqrt(var_g + eps). The sqrt is issued BEFORE the w_bf casts
    # so the dance's one ACT op isn't queued behind four big weight casts.
    sq_g = small.tile([1, G], F32)
    nc.scalar.activation(sq_g, var_g, Act.Sqrt, bias=eps11, scale=1.0)
    rstd_g = small.tile([1, G], F32)
    nc.vector.reciprocal(rstd_g, sq_g)

    # ---- weights: cast to bf16 [128(ci), 2(ci_half), 256(co)]; wot also to
    # fp8 x8 for the DoubleRow out-projection ----
    w_bf = {}
    for wi, nm in enumerate(("wqt", "wkt", "wvt", "wot")):
        wb = const.tile([P, 2, C], BF16, name=f"{nm}_bf")
        for ch in range(2):
            nc.scalar.copy(wb[:, ch, :], wall_sb[:, wi, ch, :])
        w_bf[nm] = wb
    wot8 = const.tile([P, 2, C], FP8)
    for ch in range(2):
        nc.scalar.mul(wot8[:, ch, :], wall_sb[:, 3, ch, :], 8.0)

    # broadcast group values to channels: [1,32] -> [1,256] (repeat 8) in one
    # DVE copy via a step-0 read AP
    def grp_bcast(src, name):
        dst = small.tile([1, C], F32, name=name)
        src_ap = src[:, :]
        rep = bass.AP(
            tensor=src_ap.tensor, offset=src_ap.offset,
            ap=[src_ap.ap[0], src_ap.ap[1], [0, CPG]],
        )
        nc.vector.tensor_copy(out=dst.rearrange("o (g j) -> o g j", j=CPG), in_=rep)
        return dst

    rstd_c = grp_bcast(rstd_g, "rstd_c")
    msum_c = grp_bcast(m_s, "msum_c")

    # a = gamma * rstd ; b = beta - (m_s/8) * a    (rows [1,256])
    a_row = small.tile([1, C], F32)
    nc.vector.tensor_mul(a_row, gam_row, rstd_c)
    ma_row = small.tile([1, C], F32)
    nc.vector.scalar_tensor_tensor(
        out=ma_row, in0=msum_c, scalar=1.0 / CPG, in1=a_row, op0=Alu.mult,
        op1=Alu.mult,
    )
    b_row = small.tile([1, C], F32)
    nc.vector.tensor_sub(b_row, bet_row, ma_row)

    # transpose a/b rows back to per-partition columns [128,1] per ch-half
    ab_cols = {"a": [], "b": []}
    for nm, row in (("a", a_row), ("b", b_row)):
        for ch in range(2):
            tp = ps.tile([P, 1], F32, name="tp_ab", tag="po", bufs=1)
            nc.tensor.matmul(
                tp, lhsT=row[:, ch * P:(ch + 1) * P], rhs=one11, start=True,
                stop=True,
            )
            col = small.tile([P, 1], F32, name=f"{nm}_col_{ch}")
            nc.vector.tensor_copy(out=col, in_=tp)
            ab_cols[nm].append(col)

    # ---- fold the norm affine into the projections ----
    # Q = (wq diag(a)) x_bf + (wq b + bq), same for K; V likewise with its
    # constant (wv b + bv) folded through PV/denom into bo_eff.
    b_bf = []
    for ci in range(2):
        t = small.tile([P, 1], BF16, name=f"b_bf_{ci}")
        nc.vector.tensor_copy(out=t, in_=ab_cols["b"][ci])
        b_bf.append(t)

    def matvec_bias(wname, rhs_cols, bias_add, out_dt, out_name):
        outs = []
        for co in range(2):
            pe = ps.tile([P, 1], F32, name="pe_mv", tag="po", bufs=1)
            for ci in range(2):
                nc.tensor.matmul(
                    pe, lhsT=w_bf[wname][:, ci, co * P:(co + 1) * P],
                    rhs=rhs_cols[ci], start=(ci == 0), stop=(ci == 1),
                )
            t = small.tile([P, 1], out_dt, name=f"{out_name}_{co}")
            nc.scalar.activation(
                t, pe, Act.Identity, bias=bias_add[co], scale=1.0
            )
            outs.append(t)
        return outs

    be_q = matvec_bias("wqt", b_bf, bias_cols["bq"], F32, "be_q")
    be_k = matvec_bias("wkt", b_bf, bias_cols["bk"], F32, "be_k")
    vbv_bf = matvec_bias("wvt", b_bf, bias_cols["bv"], BF16, "vbv")
    bo_eff = matvec_bias("wot", vbv_bf, bias_cols["bo"], F32, "bo_eff")
    warm(8)  # cover the be/w8 latency gap before the projections

    # scale wq/wk/wv rows by 8*a (per input channel) into fp8 tiles for the
    # DoubleRow projection matmuls; the 8x (for fp8 dynamic range on the
    # small weight values) is undone by the 1/8 in the PSUM->fp8 casts.
    a8_cols = []
    for ci in range(2):
        t = small.tile([P, 1], F32, name=f"a8_{ci}")
        nc.vector.tensor_scalar_mul(t, ab_cols["a"][ci], 8.0)
        a8_cols.append(t)
    w8 = {}
    for wname in ("wqt", "wkt", "wvt"):
        ws = const.tile([P, 2, C], FP8, name=f"{wname}_8")
        for ci in range(2):
            nc.vector.tensor_scalar_mul(
                ws[:, ci, :], w_bf[wname][:, ci, :], a8_cols[ci]
            )
        w8[wname] = ws

    # ---- projections: fp8 DoubleRow matmuls (contraction 256 = 2 ci halves
    # per instruction) over host-supplied x8, PSUM casts apply the 1/8 that
    # undoes the 8x in w8. Q/K in nb-PAIR psum tiles (4KB) so one [P, 1024]
    # cast amortizes ACT/DVE per-op overhead. ----
    k_sb = const.tile([P, 2, N], FP8)
    q_sb = const.tile([P, 2, NQ], FP8)

    def proj_pair(wname, dst, be, nbp, co):
        pp = ps.tile([P, 2, QB], F32, name="pp", tag="sps")
        for j in range(2):
            nb = 2 * nbp + j
            nc.tensor.matmul(
                pp[:, j, :], lhsT=w8[wname][:, :, co * P:(co + 1) * P],
                rhs=x8_sb[:, :, nb * QB:(nb + 1) * QB],
                start=True, stop=True, perf_mode=DR,
            )
        # tiny warm matmul: keeps the HAM activity monitor fed while the PE
        # waits on the cast engines (the phase is cast-throughput-bound)
        warm(1, cols=P)
        dcols = dst[:, co, nbp * 2 * QB:(nbp + 1) * 2 * QB]
        if co == 0:
            nc.scalar.activation(
                dcols, pp.rearrange("p j q -> p (j q)"), Act.Identity,
                bias=be[co], scale=0.125,
            )
        else:
            nc.vector.tensor_scalar(
                out=dcols, in0=pp.rearrange("p j q -> p (j q)"),
                scalar1=0.125, scalar2=be[co], op0=Alu.mult, op1=Alu.add,
            )

    for co in range(2):
        proj_pair("wqt", q_sb, be_q, 0, co)
    for nbp in range(4):
        for co in range(2):
            proj_pair("wkt", k_sb, be_k, nbp, co)
    for co in range(2):
        proj_pair("wqt", q_sb, be_q, 1, co)

    # V [N, C] fp8 (bias folded into bo_eff) in 4-chunk psum tiles:
    # psum[:, n2*C:+C] = sum_ci x8_chunk.T @ w8v
    v_sb = const.tile([P, NKT, C], FP8)
    v_flat = v_sb.rearrange("p k c -> p (k c)")
    for nt in range(0, NKT, 4):
        pv = ps.tile([P, 4 * C], F32, name="pv", tag="sps")
        for n2 in range(4):
            nc.tensor.matmul(
                pv[:, n2 * C:(n2 + 1) * C],
                lhsT=x8_sb[:, :, (nt + n2) * P:(nt + n2 + 1) * P],
                rhs=w8["wvt"][:, :, :],
                start=True, stop=True, perf_mode=DR,
            )
        warm(1, cols=P)
        if (nt // 4) % 2 == 0:
            nc.scalar.mul(v_flat[:, nt * C:(nt + 4) * C], pv, 0.125)
        else:
            nc.vector.tensor_scalar(
                out=v_flat[:, nt * C:(nt + 4) * C], in0=pv,
                scalar1=0.125, scalar2=None, op0=Alu.mult,
            )

    # ---- attention, per query block; key tiles processed in PAIRS with fp8
    # DoubleRow matmuls (contraction 256 per instruction). The softmax
    # denominator accumulates on the PE as a ones-vector DoubleRow matmul
    # into dps [1, 512]. The division is commuted through the out-projection:
    # out = (wo @ (P.V)) * (1/denom) + bo_eff + x.
    def epilogue(qb, dps, aps, last=False):
        # casts first (both DVE; scale 1/8 for the fp8 out-projection): they
        # release the PV accumulator banks immediately
        at8 = work.tile([P, 2, QB], FP8, name="at8", tag="at8", bufs=2)
        for ci in range(2):
            nc.vector.tensor_scalar(
                out=at8[:, ci, :], in0=aps[ci],
                scalar1=0.125, scalar2=None, op0=Alu.mult,
            )
        den_r = work.tile([1, QB], F32, name="den_r", tag="den_r")
        nc.vector.reciprocal_approx_fast(out=den_r, in_=dps)
        den_b = work.tile([P, QB], F32, name="den_b", tag="den_b", bufs=2)
        nc.gpsimd.partition_broadcast(den_b, den_r)
        for co in range(2):
            po = ps.tile([P, QB], F32, name="po", tag="po", bufs=1)
            nc.tensor.matmul(
                po, lhsT=wot8[:, :, co * P:(co + 1) * P],
                rhs=at8[:, :, :], start=True, stop=True, perf_mode=DR,
            )
            t1 = work.tile([P, QB], F32, name="t1", tag="t1")
            nc.vector.tensor_mul(t1, po, den_b)
            res = work.tile([P, QB], BF16, name="res", tag="res", bufs=4)
            nc.vector.scalar_tensor_tensor(
                out=res, in0=t1, scalar=bo_eff[co],
                in1=x_sb[:, co, qb * QB:(qb + 1) * QB], op0=Alu.add, op1=Alu.add,
            )
            eng = nc.sync if co == 0 else nc.scalar
            eng.dma_start(
                out=out_d[co * P:(co + 1) * P, qb * QB:(qb + 1) * QB], in_=res
            )

    pending = None
    for qb in range(NQB):
        p_sb = pblk.tile([P, NKT, QB], FP8, name="p_sb")
        dps = ps_d.tile([1, QB], F32, name="dps")
        aps = [
            ps_acc.tile([P, QB], F32, name="aps", tag="acc") for _ in range(2)
        ]
        for kp in range(NKP + 2):
            if kp == 1 and pending is not None:
                # previous qb's epilogue goes FIRST so its at_sb casts
                # precede this qb's exps in the ACT/DVE program order
                epilogue(*pending)
                pending = None
            if kp < NKP:
                sps2 = ps.tile([P, 2, QB], F32, name="sps2", tag="sps")
                for j in range(2):
                    kt = 2 * kp + j
                    nc.tensor.matmul(
                        sps2[:, j, :],
                        lhsT=k_sb[:, :, kt * P:(kt + 1) * P],
                        rhs=q_sb[:, :, qb * QB:(qb + 1) * QB],
                        start=True, stop=True, perf_mode=DR,
                    )
                nc.scalar.activation(
                    p_sb[:, 2 * kp:2 * kp + 2, :].rearrange("p k q -> p (k q)"),
                    sps2.rearrange("p k q -> p (k q)"),
                    Act.Exp, scale=SCALE, bias=shift_col,
                )
            if kp >= 2:
                pk = kp - 2
                nc.tensor.matmul(
                    dps, lhsT=ones8[:, :, 0:1],
                    rhs=p_sb[:, 2 * pk:2 * pk + 2, :],
                    start=(pk == 0), stop=(pk == NKP - 1),
                    perf_mode=DR, skip_group_check=True,
                )
                for ch in range(2):
                    nc.tensor.matmul(
                        aps[ch],
                        lhsT=v_sb[:, 2 * pk:2 * pk + 2, ch * P:(ch + 1) * P],
                        rhs=p_sb[:, 2 * pk:2 * pk + 2, :],
                        start=(pk == 0), stop=(pk == NKP - 1),
                        perf_mode=DR, skip_group_check=True,
                    )
        pending = (qb, dps, aps)
    epilogue(*pending, last=True)

    for pool in (ps_d, ps_acc, ps, work, pblk, small, const):
        pool.release()


def build_program():
    global _NC
    if _NC is not None:
        return _NC
    nc = bacc.Bacc("TRN2", target_bir_lowering=False, debug=False,
                   num_devices=NCORES)
    d = {
        "x": nc.dram_tensor("x", [C, N], BF16, kind="ExternalInput"),
        "x8": nc.dram_tensor("x8", [C, N], FP8, kind="ExternalInput"),
        "wall": nc.dram_tensor("wall", [P, 4 * 2 * C], F32, kind="ExternalInput"),
        "bq": nc.dram_tensor("bq", [C, 1], F32, kind="ExternalInput"),
        "bk": nc.dram_tensor("bk", [C, 1], F32, kind="ExternalInput"),
        "bv": nc.dram_tensor("bv", [C, 1], F32, kind="ExternalInput"),
        "bo": nc.dram_tensor("bo", [C, 1], F32, kind="ExternalInput"),
        "gamma": nc.dram_tensor("gamma", [1, C], F32, kind="ExternalInput"),
        "beta": nc.dram_tensor("beta", [1, C], F32, kind="ExternalInput"),
        "out": nc.dram_tensor("out", [C, NQ], BF16, kind="ExternalOutput"),
    }
    with tile.TileContext(nc) as tc:
        _body(tc, d)
    nc.compile()
    _NC = nc
    return nc


def make_in_maps(x, gamma, beta, wq, bq, wk, bk, wv, bv, wo, bo):
    f32c = lambda a: np.ascontiguousarray(np.asarray(a, dtype=np.float32))
    x = f32c(x)
    # wall[p, k, h, co] = w_k^T[h*128+p, co]  (k in {q,k,v,o})
    wall = np.stack([
        np.asarray(w, np.float32).T for w in (wq, wk, wv, wo)
    ]).reshape(4, 2, P, C).transpose(2, 0, 1, 3).reshape(P, 4 * 2 * C)
    base = {
        "wall": f32c(wall),
        "bq": f32c(bq).reshape(C, 1),
        "bk": f32c(bk).reshape(C, 1),
        "bv": f32c(bv).reshape(C, 1),
        "bo": f32c(bo).reshape(C, 1),
        "gamma": f32c(gamma).reshape(1, C),
        "beta": f32c(beta).reshape(1, C),
    }
    import ml_dtypes

    in_maps = []
    for core in range(NCORES):
        b, h = divmod(core, 2)
        xb = x[b].reshape(C, N)
        if h:
            xb = np.concatenate([xb[:, NQ:], xb[:, :NQ]], axis=1)
        in_maps.append({
            **base,
            "x": np.ascontiguousarray(xb.astype(ml_dtypes.bfloat16)),
            "x8": np.ascontiguousarray(xb.astype(ml_dtypes.float8_e4m3)),
        })
    return in_maps


def kernel(x, gamma, beta, wq, bq, wk, bk, wv, bv, wo, bo):
    global LAST_RESULTS
    from concourse.bass_utils import run_bass_kernel_spmd

    nc = build_program()
    in_maps = make_in_maps(x, gamma, beta, wq, bq, wk, bk, wv, bv, wo, bo)
    res = run_bass_kernel_spmd(nc, in_maps, core_ids=list(range(NCORES)))
    LAST_RESULTS = res
    out = np.empty((B, C, N), np.float32)
    for core in range(NCORES):
        b, h = divmod(core, 2)
        out[b][:, h * NQ:(h + 1) * NQ] = np.asarray(
            res.results[core]["out"], dtype=np.float32
        )
    return out.reshape(B, C, H, W)


# revision 42
# speedup vs baseline: 1.2463x; 1.2334x over previous
"""AttnBlock (GroupNorm + single-head self-attention + residual) on 8 TRN2 cores.

Sharding: data-parallel over (batch b, query-half h) -> 8 shards. Each core
receives the full [C, N] image of its batch (columns rolled so that its own
query half always occupies columns 0:NQ), computes GroupNorm stats + K/V over
the whole image, Q over its half, and a flash-style attention in which scores
are produced directly transposed (S^T = K^T.T @ Q^T tiles) so softmax
normalization needs no PE transposes of P.

The attention inner loop runs in fp8e4 with DoubleRow perf mode (2 contraction
rows per PE cell): one S matmul per key tile (contraction 256 = 2x128 channel
halves), PV over key-tile pairs, and the softmax denominator as a ones-vector
DoubleRow matmul accumulated into a [1, 512] PSUM row. exp() is applied to
key-tile PAIRS ([128, 1024] activations) to amortize ACT overhead, shifted by
-SHIFT so exp output fits fp8e4's +/-240 range (scores reach ~8).
Projections and the out-projection stay bf16 for accuracy.
"""

import os
import sys

import numpy as np

for _p in ("/opt/trn_rl_repo", "/root/.axon_site/_ro/trn_rl_repo"):
    if os.path.isdir(_p) and _p not in sys.path:
        sys.path.insert(0, _p)

import concourse.bass as bass  # noqa: E402
import concourse.tile as tile  # noqa: E402
from concourse import bacc, mybir  # noqa: E402
from concourse.masks import make_identity  # noqa: E402

# The agent image's antenv lacks axon_hooks; if BASS_TRACE is set in the
# environment, run_bass_kernel_spmd would crash importing it. Provide a stub
# (profiling degrades gracefully to "hook isn't registered").
try:
    import antenv.axon_hooks  # noqa: F401
except ImportError:
    import types as _types

    _m = _types.ModuleType("antenv.axon_hooks")
    _h = [None]
    _m.set_axon_ntff_profile_hook = lambda h: _h.__setitem__(0, h)
    _m.get_axon_ntff_profile_hook = lambda: _h[0]
    sys.modules["antenv.axon_hooks"] = _m

B, C, H, W = 4, 256, 64, 64
N = H * W  # 4096 pixels
NQ = N // 2  # 2048 queries per core
G = 32  # groups
CPG = C // G  # 8 channels per group
EPS = 1e-5
NCORES = 8
SCALE = float(C) ** -0.5  # 0.0625
SHIFT = 4.0  # exp(s - SHIFT): keeps exp <= ~50 << fp8e4 max 240

F32 = mybir.dt.float32
BF16 = mybir.dt.bfloat16
FP8 = mybir.dt.float8e4

QB = 512  # query block (free dim of S^T / PV matmuls)
NQB = NQ // QB  # 4 query blocks
NKT = N // 128  # 32 key tiles
NKP = NKT // 2  # 16 key-tile pairs
NNB = N // QB  # 8 pixel blocks for K/V projections
P = 128

Act = mybir.ActivationFunctionType
Alu = mybir.AluOpType
Axis = mybir.AxisListType
DR = mybir.MatmulPerfMode.DoubleRow

_NC = None
LAST_RESULTS = None


def _body(tc, d):
    nc = tc.nc
    x_d = d["x"]
    out_d = d["out"]

    const = tc.alloc_tile_pool(name="const", bufs=1)
    small = tc.alloc_tile_pool(name="small", bufs=1)
    pblk = tc.alloc_tile_pool(name="pblk", bufs=2)
    work = tc.alloc_tile_pool(name="work", bufs=2)
    # PSUM budget (8 banks): "sps" 4KB x2 = 4, "po" 2KB x1 = 1, acc 2, dps 1
    ps = tc.alloc_tile_pool(name="ps", bufs=2, space="PSUM")
    ps_acc = tc.alloc_tile_pool(name="ps_acc", bufs=2, space="PSUM")
    ps_d = tc.alloc_tile_pool(name="ps_d", bufs=1, space="PSUM")

    # ---- constants issued first so every engine's stream opens with
    # dependency-free work (PE warm-up, ACT table preload) ----
    wu_w = const.tile([P, P], BF16)
    nc.vector.memset(wu_w, 0.0)
    wu_x = const.tile([P, QB], BF16)
    nc.vector.memset(wu_x, 0.0)
    wu_ps = ps.tile([P, QB], F32, name="wu_ps", tag="po", bufs=1)

    def warm(n, cols=QB):
        for _ in range(n):
            nc.tensor.matmul(
                wu_ps[:, 0:cols], lhsT=wu_w, rhs=wu_x[:, 0:cols],
                start=True, stop=True,
            )

    ident = const.tile([P, P], F32)
    make_identity(nc, ident)
    one11 = const.tile([1, 1], F32)
    nc.vector.memset(one11, 1.0)
    ones_f = const.tile([P, 1], F32)
    nc.vector.memset(ones_f, 1.0)
    # fp8 "ones" pair for the denominator DoubleRow matmul. Padded free dim so
    # the pair-dim byte step is 16 (ISA requires step % 16 == 0).
    ones8 = const.tile([P, 2, 16], FP8)
    nc.vector.memset(ones8, 1.0)
    eps11 = const.tile([1, 1], F32)
    nc.vector.memset(eps11, EPS)
    shift_col = const.tile([P, 1], F32)
    nc.vector.memset(shift_col, -SHIFT)

    # ---- bulk DMAs first, on the two hardware DGE queues (sync + scalar).
    # Weights are host-packed into one [128, 2048] f32 tensor (8KB contiguous
    # per partition line); x arrives twice from host: bf16 (stats + residual)
    # and fp8 (projection matmul operand). x halves use 4KB lines so bn_stats
    # can start on the first half while the rest streams in. ----
    x_sb = const.tile([P, 2, N], BF16)
    x_bf = x_sb
    x8_sb = const.tile([P, 2, N], FP8)
    wall_sb = const.tile([P, 4, 2, C], F32)
    x_src = x_d.ap().rearrange("(h p) n -> p h n", p=P)
    x8_src = d["x8"].ap().rearrange("(h p) n -> p h n", p=P)
    for half in range(2):
        for ch in range(2):
            sl = (slice(None), ch, slice(half * 4 * QB, (half + 1) * 4 * QB))
            eng = nc.sync if ch == 0 else nc.scalar
            eng.dma_start(out=x_sb[sl], in_=x_src[sl])
    # x8 on sync (needed only at projection time), wall after x on scalar
    for ch in range(2):
        nc.sync.dma_start(
            out=x8_sb[:, ch, :], in_=x8_src[:, ch, :]
        )
    nc.scalar.dma_start(
        out=wall_sb.rearrange("p k h c -> p (k h c)"), in_=d["wall"].ap()
    )

    bn_st = [small.tile([P, NNB, 6], F32, name=f"bnst_{ch}") for ch in range(2)]
    for half in range(2):
        for ch in range(2):
            for j in range(4):
                jj = half * 4 + j
                nc.vector.bn_stats(
                    out=bn_st[ch][:, jj, :],
                    in_=x_sb[:, ch, jj * QB:(jj + 1) * QB],
                )

    # PE warm-up: keep the HAM activity monitor busy until projections start.
    # The plain warms run back-to-back from t~8us; the x-slice warms DEPEND on
    # the arriving DMA chunks, so they spread across the stats window and keep
    # feeding the activity monitor while the GroupNorm dance runs.
    warm(36)
    for half in range(2):
        for ch in range(2):
            for r in range(3):
                nc.tensor.matmul(
                    wu_ps, lhsT=wu_w,
                    rhs=x_sb[:, ch, (half * 4 + r) * QB:(half * 4 + r + 1) * QB],
                    start=True, stop=True,
                )

    # preload the ACT exp and sqrt tables before the dance/attention need them
    # (issued after the scalar-queue DMAs so they don't delay the transfers)
    warm11 = small.tile([1, 1], F32)
    nc.scalar.activation(warm11, one11, Act.Exp, scale=1.0)
    warm12 = small.tile([1, 1], F32)
    nc.scalar.activation(warm12, one11, Act.Sqrt, scale=1.0)

    gam_row = const.tile([1, C], F32)
    nc.gpsimd.dma_start(out=gam_row, in_=d["gamma"][:, :])
    bet_row = const.tile([1, C], F32)
    nc.gpsimd.dma_start(out=bet_row, in_=d["beta"][:, :])

    # per-partition bias columns [128,1] x 2 channel-halves
    bias_cols = {}
    for nm in ("bq", "bk", "bv", "bo"):
        cols = []
        for ch in range(2):
            t = const.tile([P, 1], F32, name=f"{nm}_{ch}")
            nc.gpsimd.dma_start(out=t, in_=d[nm][ch * P:(ch + 1) * P, :])
            cols.append(t)
        bias_cols[nm] = cols

    # ---- GroupNorm statistics (bn_stats already issued in the DMA loop) ----
    mv = []
    for ch in range(2):
        m = small.tile([P, 2], F32, name=f"mv_{ch}")
        nc.vector.bn_aggr(out=m, in_=bn_st[ch])
        mv.append(m)

    # transpose per-channel mean and var into one row [1, 512]:
    # [mean_c0 | mean_c1 | var_c0 | var_c1]
    tp = ps.tile([1, 4 * P], F32, name="tp_stat", tag="po", bufs=1)
    for ch in range(2):
        for k in range(2):
            nc.tensor.transpose(
                tp[:, (2 * k + ch) * P:(2 * k + ch + 1) * P], mv[ch][:, k:k + 1],
                ident,
            )
    mrows = small.tile([1, 4 * P], F32)
    nc.vector.tensor_copy(out=mrows, in_=tp)
    mean_row = mrows[:, 0:C]
    var_row = mrows[:, C:2 * C]

    warm(8)  # keep the PE activity monitor warm through the stats dance

    # group stats (sums over the 8 channels of each group)
    msq_row = small.tile([1, C], F32)
    nc.vector.tensor_mul(msq_row, mean_row, mean_row)
    ex2_row = small.tile([1, C], F32)
    nc.vector.tensor_add(ex2_row, msq_row, var_row)
    m_s = small.tile([1, G], F32)
    nc.vector.tensor_reduce(
        out=m_s, in_=mean_row.rearrange("o (g j) -> o g j", j=CPG), axis=Axis.X,
        op=Alu.add,
    )
    e_s = small.tile([1, G], F32)
    nc.vector.tensor_reduce(
        out=e_s, in_=ex2_row.rearrange("o (g j) -> o g j", j=CPG), axis=Axis.X,
        op=Alu.add,
    )
    # var_g = e_s/8 - (m_s/8)^2
    mm_g = small.tile([1, G], F32)
    nc.vector.tensor_mul(mm_g, m_s, m_s)
    mm_g2 = small.tile([1, G], F32)
    nc.vector.tensor_scalar_mul(mm_g2, mm_g, 1.0 / (CPG * CPG))
    var_g = small.tile([1, G], F32)
    nc.vector.scalar_tensor_tensor(
        out=var_g, in0=e_s, scalar=1.0 / CPG, in1=mm_g2, op0=Alu.mult,
        op1=Alu.subtract,
    )
    # rstd_g = 1/sqrt(var_g + eps). The sqrt is issued BEFORE the w_bf casts
    # so the dance's one ACT op isn't queued behind four big weight casts.
    sq_g = small.tile([1, G], F32)
    nc.scalar.activation(sq_g, var_g, Act.Sqrt, bias=eps11, scale=1.0)
    rstd_g = small.tile([1, G], F32)
    nc.vector.reciprocal(rstd_g, sq_g)

    # ---- weights: cast to bf16 [128(ci), 2(ci_half), 256(co)]; wot also to
    # fp8 x8 for the DoubleRow out-projection ----
    w_bf = {}
    for wi, nm in enumerate(("wqt", "wkt", "wvt", "wot")):
        wb = const.tile([P, 2, C], BF16, name=f"{nm}_bf")
        for ch in range(2):
            nc.scalar.copy(wb[:, ch, :], wall_sb[:, wi, ch, :])
        w_bf[nm] = wb
    wot8 = const.tile([P, 2, C], FP8)
    for ch in range(2):
        nc.scalar.mul(wot8[:, ch, :], wall_sb[:, 3, ch, :], 8.0)

    # broadcast group values to channels: [1,32] -> [1,256] (repeat 8) in one
    # DVE copy via a step-0 read AP
    def grp_bcast(src, name):
        dst = small.tile([1, C], F32, name=name)
        src_ap = src[:, :]
        rep = bass.AP(
            tensor=src_ap.tensor, offset=src_ap.offset,
            ap=[src_ap.ap[0], src_ap.ap[1], [0, CPG]],
        )
        nc.vector.tensor_copy(out=dst.rearrange("o (g j) -> o g j", j=CPG), in_=rep)
        return dst

    rstd_c = grp_bcast(rstd_g, "rstd_c")
    msum_c = grp_bcast(m_s, "msum_c")

    # a = gamma * rstd ; b = beta - (m_s/8) * a    (rows [1,256])
    a_row = small.tile([1, C], F32)
    nc.vector.tensor_mul(a_row, gam_row, rstd_c)
    ma_row = small.tile([1, C], F32)
    nc.vector.scalar_tensor_tensor(
        out=ma_row, in0=msum_c, scalar=1.0 / CPG, in1=a_row, op0=Alu.mult,
        op1=Alu.mult,
    )
    b_row = small.tile([1, C], F32)
    nc.vector.tensor_sub(b_row, bet_row, ma_row)

    # transpose a/b rows back to per-partition columns [128,1] per ch-half
    ab_cols = {"a": [], "b": []}
    for nm, row in (("a", a_row), ("b", b_row)):
        for ch in range(2):
            tp = ps.tile([P, 1], F32, name="tp_ab", tag="po", bufs=1)
            nc.tensor.matmul(
                tp, lhsT=row[:, ch * P:(ch + 1) * P], rhs=one11, start=True,
                stop=True,
            )
            col = small.tile([P, 1], F32, name=f"{nm}_col_{ch}")
            nc.vector.tensor_copy(out=col, in_=tp)
            ab_cols[nm].append(col)

    # ---- fold the norm affine into the projections ----
    # Q = (wq diag(a)) x_bf + (wq b + bq), same for K; V likewise with its
    # constant (wv b + bv) folded through PV/denom into bo_eff.
    b_bf = []
    for ci in range(2):
        t = small.tile([P, 1], BF16, name=f"b_bf_{ci}")
        nc.vector.tensor_copy(out=t, in_=ab_cols["b"][ci])
        b_bf.append(t)

    def matvec_bias(wname, rhs_cols, bias_add, out_dt, out_name):
        outs = []
        for co in range(2):
            pe = ps.tile([P, 1], F32, name="pe_mv", tag="po", bufs=1)
            for ci in range(2):
                nc.tensor.matmul(
                    pe, lhsT=w_bf[wname][:, ci, co * P:(co + 1) * P],
                    rhs=rhs_cols[ci], start=(ci == 0), stop=(ci == 1),
                )
            t = small.tile([P, 1], out_dt, name=f"{out_name}_{co}")
            nc.scalar.activation(
                t, pe, Act.Identity, bias=bias_add[co], scale=1.0
            )
            outs.append(t)
        return outs

    be_q = matvec_bias("wqt", b_bf, bias_cols["bq"], F32, "be_q")
    be_k = matvec_bias("wkt", b_bf, bias_cols["bk"], F32, "be_k")
    vbv_bf = matvec_bias("wvt", b_bf, bias_cols["bv"], BF16, "vbv")
    bo_eff = matvec_bias("wot", vbv_bf, bias_cols["bo"], F32, "bo_eff")
    warm(8)  # cover the be/w8 latency gap before the projections

    # scale wq/wk/wv rows by 8*a (per input channel) into fp8 tiles for the
    # DoubleRow projection matmuls; the 8x (for fp8 dynamic range on the
    # small weight values) is undone by the 1/8 in the PSUM->fp8 casts.
    # Split across DVE and ACT (per-partition AP scale) so wqt is ready
    # ~800ns after the a8 columns and the Q projection starts immediately.
    a8_cols = []
    for ci in range(2):
        t = small.tile([P, 1], F32, name=f"a8_{ci}")
        nc.vector.tensor_scalar_mul(t, ab_cols["a"][ci], 8.0)
        a8_cols.append(t)
    w8 = {}
    for wname in ("wqt", "wkt", "wvt"):
        ws = const.tile([P, 2, C], FP8, name=f"{wname}_8")
        nc.vector.tensor_scalar_mul(ws[:, 0, :], w_bf[wname][:, 0, :], a8_cols[0])
        nc.scalar.activation(
            ws[:, 1, :], w_bf[wname][:, 1, :], Act.Copy, bias=0.0,
            scale=a8_cols[1],
        )
        w8[wname] = ws

    # ---- projections: fp8 DoubleRow matmuls (contraction 256 = 2 ci halves
    # per instruction) over host-supplied x8, PSUM casts apply the 1/8 that
    # undoes the 8x in w8. Q/K in nb-PAIR psum tiles (4KB) so one [P, 1024]
    # cast amortizes ACT/DVE per-op overhead. ----
    k_sb = const.tile([P, 2, N], FP8)
    q_sb = const.tile([P, 2, NQ], FP8)

    def proj_pair(wname, dst, be, nbp, co):
        pp = ps.tile([P, 2, QB], F32, name="pp", tag="sps")
        for j in range(2):
            nb = 2 * nbp + j
            nc.tensor.matmul(
                pp[:, j, :], lhsT=w8[wname][:, :, co * P:(co + 1) * P],
                rhs=x8_sb[:, :, nb * QB:(nb + 1) * QB],
                start=True, stop=True, perf_mode=DR,
            )
        # tiny warm matmul: keeps the HAM activity monitor fed while the PE
        # waits on the cast engines (the phase is cast-throughput-bound)
        warm(1, cols=P)
        dcols = dst[:, co, nbp * 2 * QB:(nbp + 1) * 2 * QB]
        if co == 0:
            nc.scalar.activation(
                dcols, pp.rearrange("p j q -> p (j q)"), Act.Identity,
                bias=be[co], scale=0.125,
            )
        else:
            nc.vector.tensor_scalar(
                out=dcols, in0=pp.rearrange("p j q -> p (j q)"),
                scalar1=0.125, scalar2=be[co], op0=Alu.mult, op1=Alu.add,
            )

    for co in range(2):
        proj_pair("wqt", q_sb, be_q, 0, co)
    for nbp in range(4):
        for co in range(2):
            proj_pair("wkt", k_sb, be_k, nbp, co)
    for co in range(2):
        proj_pair("wqt", q_sb, be_q, 1, co)

    # V [N, C] fp8 (bias folded into bo_eff) in 4-chunk psum tiles:
    # psum[:, n2*C:+C] = sum_ci x8_chunk.T @ w8v
    v_sb = const.tile([P, NKT, C], FP8)
    v_flat = v_sb.rearrange("p k c -> p (k c)")
    for nt in range(0, NKT, 4):
        pv = ps.tile([P, 4 * C], F32, name="pv", tag="sps")
        for n2 in range(4):
            nc.tensor.matmul(
                pv[:, n2 * C:(n2 + 1) * C],
                lhsT=x8_sb[:, :, (nt + n2) * P:(nt + n2 + 1) * P],
                rhs=w8["wvt"][:, :, :],
                start=True, stop=True, perf_mode=DR,
            )
        warm(1, cols=P)
        if (nt // 4) % 2 == 0:
            nc.scalar.mul(v_flat[:, nt * C:(nt + 4) * C], pv, 0.125)
        else:
            nc.vector.tensor_scalar(
                out=v_flat[:, nt * C:(nt + 4) * C], in0=pv,
                scalar1=0.125, scalar2=None, op0=Alu.mult,
            )

    # ---- attention, per query block; key tiles processed in PAIRS with fp8
    # DoubleRow matmuls (contraction 256 per instruction). The softmax
    # denominator accumulates on the PE as a ones-vector DoubleRow matmul
    # into dps [1, 512]. The division is commuted through the out-projection:
    # out = (wo @ (P.V)) * (1/denom) + bo_eff + x.
    def epilogue(qb, dps, aps, last=False):
        # casts first (both DVE; scale 1/8 for the fp8 out-projection): they
        # release the PV accumulator banks immediately
        at8 = work.tile([P, 2, QB], FP8, name="at8", tag="at8", bufs=2)
        for ci in range(2):
            nc.vector.tensor_scalar(
                out=at8[:, ci, :], in0=aps[ci],
                scalar1=0.125, scalar2=None, op0=Alu.mult,
            )
        den_r = work.tile([1, QB], F32, name="den_r", tag="den_r")
        nc.vector.reciprocal_approx_fast(out=den_r, in_=dps)
        den_b = work.tile([P, QB], F32, name="den_b", tag="den_b", bufs=2)
        nc.gpsimd.partition_broadcast(den_b, den_r)
        for co in range(2):
            po = ps.tile([P, QB], F32, name="po", tag="po", bufs=1)
            nc.tensor.matmul(
                po, lhsT=wot8[:, :, co * P:(co + 1) * P],
                rhs=at8[:, :, :], start=True, stop=True, perf_mode=DR,
            )
            t1 = work.tile([P, QB], F32, name="t1", tag="t1")
            nc.vector.tensor_mul(t1, po, den_b)
            res = work.tile([P, QB], BF16, name="res", tag="res", bufs=4)
            nc.vector.scalar_tensor_tensor(
                out=res, in0=t1, scalar=bo_eff[co],
                in1=x_sb[:, co, qb * QB:(qb + 1) * QB], op0=Alu.add, op1=Alu.add,
            )
            eng = nc.sync if co == 0 else nc.scalar
            eng.dma_start(
                out=out_d[co * P:(co + 1) * P, qb * QB:(qb + 1) * QB], in_=res
            )

    pending = None
    for qb in range(NQB):
        p_sb = pblk.tile([P, NKT, QB], FP8, name="p_sb")
        dps = ps_d.tile([1, QB], F32, name="dps")
        aps = [
            ps_acc.tile([P, QB], F32, name="aps", tag="acc") for _ in range(2)
        ]
        for kp in range(NKP + 2):
            if kp == 1 and pending is not None:
                # previous qb's epilogue goes FIRST so its at_sb casts
                # precede this qb's exps in the ACT/DVE program order
                epilogue(*pending)
                pending = None
            if kp < NKP:
                sps2 = ps.tile([P, 2, QB], F32, name="sps2", tag="sps")
                for j in range(2):
                    kt = 2 * kp + j
                    nc.tensor.matmul(
                        sps2[:, j, :],
                        lhsT=k_sb[:, :, kt * P:(kt + 1) * P],
                        rhs=q_sb[:, :, qb * QB:(qb + 1) * QB],
                        start=True, stop=True, perf_mode=DR,
                    )
                nc.scalar.activation(
                    p_sb[:, 2 * kp:2 * kp + 2, :].rearrange("p k q -> p (k q)"),
                    sps2.rearrange("p k q -> p (k q)"),
                    Act.Exp, scale=SCALE, bias=shift_col,
                )
            if kp >= 2:
                pk = kp - 2
                nc.tensor.matmul(
                    dps, lhsT=ones8[:, :, 0:1],
                    rhs=p_sb[:, 2 * pk:2 * pk + 2, :],
                    start=(pk == 0), stop=(pk == NKP - 1),
                    perf_mode=DR, skip_group_check=True,
                )
                for ch in range(2):
                    nc.tensor.matmul(
                        aps[ch],
                        lhsT=v_sb[:, 2 * pk:2 * pk + 2, ch * P:(ch + 1) * P],
                        rhs=p_sb[:, 2 * pk:2 * pk + 2, :],
                        start=(pk == 0), stop=(pk == NKP - 1),
                        perf_mode=DR, skip_group_check=True,
                    )
        pending = (qb, dps, aps)
    epilogue(*pending, last=True)

    for pool in (ps_d, ps_acc, ps, work, pblk, small, const):
        pool.release()


def build_program():
    global _NC
    if _NC is not None:
        return _NC
    nc = bacc.Bacc("TRN2", target_bir_lowering=False, debug=False,
                   num_devices=NCORES)
    d = {
        "x": nc.dram_tensor("x", [C, N], BF16, kind="ExternalInput"),
        "x8": nc.dram_tensor("x8", [C, N], FP8, kind="ExternalInput"),
        "wall": nc.dram_tensor("wall", [P, 4 * 2 * C], F32, kind="ExternalInput"),
        "bq": nc.dram_tensor("bq", [C, 1], F32, kind="ExternalInput"),
        "bk": nc.dram_tensor("bk", [C, 1], F32, kind="ExternalInput"),
        "bv": nc.dram_tensor("bv", [C, 1], F32, kind="ExternalInput"),
        "bo": nc.dram_tensor("bo", [C, 1], F32, kind="ExternalInput"),
        "gamma": nc.dram_tensor("gamma", [1, C], F32, kind="ExternalInput"),
        "beta": nc.dram_tensor("beta", [1, C], F32, kind="ExternalInput"),
        "out": nc.dram_tensor("out", [C, NQ], BF16, kind="ExternalOutput"),
    }
    with tile.TileContext(nc) as tc:
        _body(tc, d)
    nc.compile()
    _NC = nc
    return nc


def make_in_maps(x, gamma, beta, wq, bq, wk, bk, wv, bv, wo, bo):
    f32c = lambda a: np.ascontiguousarray(np.asarray(a, dtype=np.float32))
    x = f32c(x)
    # wall[p, k, h, co] = w_k^T[h*128+p, co]  (k in {q,k,v,o})
    wall = np.stack([
        np.asarray(w, np.float32).T for w in (wq, wk, wv, wo)
    ]).reshape(4, 2, P, C).transpose(2, 0, 1, 3).reshape(P, 4 * 2 * C)
    base = {
        "wall": f32c(wall),
        "bq": f32c(bq).reshape(C, 1),
        "bk": f32c(bk).reshape(C, 1),
        "bv": f32c(bv).reshape(C, 1),
        "bo": f32c(bo).reshape(C, 1),
        "gamma": f32c(gamma).reshape(1, C),
        "beta": f32c(beta).reshape(1, C),
    }
    import ml_dtypes

    in_maps = []
    for core in range(NCORES):
        b, h = divmod(core, 2)
        xb = x[b].reshape(C, N)
        if h:
            xb = np.concatenate([xb[:, NQ:], xb[:, :NQ]], axis=1)
        in_maps.append({
            **base,
            "x": np.ascontiguousarray(xb.astype(ml_dtypes.bfloat16)),
            "x8": np.ascontiguousarray(xb.astype(ml_dtypes.float8_e4m3)),
        })
    return in_maps


def kernel(x, gamma, beta, wq, bq, wk, bk, wv, bv, wo, bo):
    global LAST_RESULTS
    from concourse.bass_utils import run_bass_kernel_spmd

    nc = build_program()
    in_maps = make_in_maps(x, gamma, beta, wq, bq, wk, bk, wv, bv, wo, bo)
    res = run_bass_kernel_spmd(nc, in_maps, core_ids=list(range(NCORES)))
    LAST_RESULTS = res
    out = np.empty((B, C, N), np.float32)
    for core in range(NCORES):
        b, h = divmod(core, 2)
        out[b][:, h * NQ:(h + 1) * NQ] = np.asarray(
            res.results[core]["out"], dtype=np.float32
        )
    return out.reshape(B, C, H, W)


# revision 48
# speedup vs baseline: 1.2625x; 1.0131x over previous
"""AttnBlock (GroupNorm + single-head self-attention + residual) on 8 TRN2 cores.

Sharding: data-parallel over (batch b, query-half h) -> 8 shards. Each core
receives the full [C, N] image of its batch (columns rolled so that its own
query half always occupies columns 0:NQ), computes GroupNorm stats + K/V over
the whole image, Q over its half, and a flash-style attention in which scores
are produced directly transposed (S^T = K^T.T @ Q^T tiles) so softmax
normalization needs no PE transposes of P.

The attention inner loop runs in fp8e4 with DoubleRow perf mode (2 contraction
rows per PE cell): one S matmul per key tile (contraction 256 = 2x128 channel
halves), PV over key-tile pairs, and the softmax denominator as a ones-vector
DoubleRow matmul accumulated into a [1, 512] PSUM row. exp() is applied to
key-tile PAIRS ([128, 1024] activations) to amortize ACT overhead, shifted by
-SHIFT so exp output fits fp8e4's +/-240 range (scores reach ~8).
Projections and the out-projection stay bf16 for accuracy.
"""

import os
import sys

import numpy as np

for _p in ("/opt/trn_rl_repo", "/root/.axon_site/_ro/trn_rl_repo"):
    if os.path.isdir(_p) and _p not in sys.path:
        sys.path.insert(0, _p)

import concourse.bass as bass  # noqa: E402
import concourse.tile as tile  # noqa: E402
from concourse import bacc, mybir  # noqa: E402
from concourse.masks import make_identity  # noqa: E402

# The agent image's antenv lacks axon_hooks; if BASS_TRACE is set in the
# environment, run_bass_kernel_spmd would crash importing it. Provide a stub
# (profiling degrades gracefully to "hook isn't registered").
try:
    import antenv.axon_hooks  # noqa: F401
except ImportError:
    import types as _types

    _m = _types.ModuleType("antenv.axon_hooks")
    _h = [None]
    _m.set_axon_ntff_profile_hook = lambda h: _h.__setitem__(0, h)
    _m.get_axon_ntff_profile_hook = lambda: _h[0]
    sys.modules["antenv.axon_hooks"] = _m

B, C, H, W = 4, 256, 64, 64
N = H * W  # 4096 pixels
NQ = N // 2  # 2048 queries per core
G = 32  # groups
CPG = C // G  # 8 channels per group
EPS = 1e-5
NCORES = 8
SCALE = float(C) ** -0.5  # 0.0625
SHIFT = 4.0  # exp(s - SHIFT): keeps exp <= ~50 << fp8e4 max 240

F32 = mybir.dt.float32
BF16 = mybir.dt.bfloat16
FP8 = mybir.dt.float8e4

QB = 512  # query block (free dim of S^T / PV matmuls)
NQB = NQ // QB  # 4 query blocks
NKT = N // 128  # 32 key tiles
NKP = NKT // 2  # 16 key-tile pairs
NNB = N // QB  # 8 pixel blocks for K/V projections
P = 128

Act = mybir.ActivationFunctionType
Alu = mybir.AluOpType
Axis = mybir.AxisListType
DR = mybir.MatmulPerfMode.DoubleRow

_NC = None
LAST_RESULTS = None


def _body(tc, d):
    nc = tc.nc
    x_d = d["x"]
    out_d = d["out"]

    const = tc.alloc_tile_pool(name="const", bufs=1)
    small = tc.alloc_tile_pool(name="small", bufs=1)
    pblk = tc.alloc_tile_pool(name="pblk", bufs=2)
    work = tc.alloc_tile_pool(name="work", bufs=2)
    # PSUM budget (8 banks): "sps" 4KB x2 = 4, "po" 2KB x1 = 1, acc 2, dps 1
    ps = tc.alloc_tile_pool(name="ps", bufs=2, space="PSUM")
    ps_acc = tc.alloc_tile_pool(name="ps_acc", bufs=2, space="PSUM")
    ps_d = tc.alloc_tile_pool(name="ps_d", bufs=1, space="PSUM")

    # ---- constants issued first so every engine's stream opens with
    # dependency-free work (PE warm-up, ACT table preload) ----
    wu_w = const.tile([P, P], BF16)
    nc.vector.memset(wu_w, 0.0)
    wu_x = const.tile([P, QB], BF16)
    nc.vector.memset(wu_x, 0.0)
    # warm-up PSUM lives in the (attention-only) dps slot so warm matmuls
    # never serialize against the dance/epilogue tiles in the "po" ring
    wu_ps = ps_d.tile([P, QB], F32, name="wu_ps", tag="dps", bufs=1)

    def warm(n, cols=QB):
        for _ in range(n):
            nc.tensor.matmul(
                wu_ps[:, 0:cols], lhsT=wu_w, rhs=wu_x[:, 0:cols],
                start=True, stop=True,
            )

    ident = const.tile([P, P], F32)
    make_identity(nc, ident)
    one11 = const.tile([1, 1], F32)
    nc.vector.memset(one11, 1.0)
    ones_f = const.tile([P, 1], F32)
    nc.vector.memset(ones_f, 1.0)
    # fp8 "ones" pair for the denominator DoubleRow matmul. Padded free dim so
    # the pair-dim byte step is 16 (ISA requires step % 16 == 0).
    ones8 = const.tile([P, 2, 16], FP8)
    nc.vector.memset(ones8, 1.0)
    eps11 = const.tile([1, 1], F32)
    nc.vector.memset(eps11, EPS)
    shift_col = const.tile([P, 1], F32)
    nc.vector.memset(shift_col, -SHIFT)

    # ---- bulk DMAs first, on the two hardware DGE queues (sync + scalar).
    # Weights are host-packed into one [128, 2048] f32 tensor (8KB contiguous
    # per partition line); x arrives twice from host: bf16 (stats + residual)
    # and fp8 (projection matmul operand). x halves use 4KB lines so bn_stats
    # can start on the first half while the rest streams in. ----
    x_sb = const.tile([P, 2, N], BF16)
    x_bf = x_sb
    x8_sb = const.tile([P, 2, N], FP8)
    wall_sb = const.tile([P, 4, 2, C], F32)
    x_src = x_d.ap().rearrange("(h p) n -> p h n", p=P)
    x8_src = d["x8"].ap().rearrange("(h p) n -> p h n", p=P)
    for half in range(2):
        for ch in range(2):
            sl = (slice(None), ch, slice(half * 4 * QB, (half + 1) * 4 * QB))
            eng = nc.sync if ch == 0 else nc.scalar
            eng.dma_start(out=x_sb[sl], in_=x_src[sl])
    # x8 on sync (needed only at projection time), wall after x on scalar
    for ch in range(2):
        nc.sync.dma_start(
            out=x8_sb[:, ch, :], in_=x8_src[:, ch, :]
        )
    nc.scalar.dma_start(
        out=wall_sb.rearrange("p k h c -> p (k h c)"), in_=d["wall"].ap()
    )

    bn_st = [small.tile([P, NNB, 6], F32, name=f"bnst_{ch}") for ch in range(2)]
    for half in range(2):
        for ch in range(2):
            for j in range(4):
                jj = half * 4 + j
                nc.vector.bn_stats(
                    out=bn_st[ch][:, jj, :],
                    in_=x_sb[:, ch, jj * QB:(jj + 1) * QB],
                )

    # PE warm-up: keep the HAM activity monitor busy until projections start.
    # The plain warms run back-to-back from t~8us; the x-slice warms DEPEND on
    # the arriving DMA chunks, so they spread across the stats window and keep
    # feeding the activity monitor while the GroupNorm dance runs.
    warm(36)
    for half in range(2):
        for ch in range(2):
            for r in range(3):
                nc.tensor.matmul(
                    wu_ps, lhsT=wu_w,
                    rhs=x_sb[:, ch, (half * 4 + r) * QB:(half * 4 + r + 1) * QB],
                    start=True, stop=True,
                )

    # preload the ACT exp and sqrt tables before the dance/attention need them
    # (issued after the scalar-queue DMAs so they don't delay the transfers)
    warm11 = small.tile([1, 1], F32)
    nc.scalar.activation(warm11, one11, Act.Exp, scale=1.0)
    warm12 = small.tile([1, 1], F32)
    nc.scalar.activation(warm12, one11, Act.Sqrt, scale=1.0)

    gam_row = const.tile([1, C], F32)
    nc.gpsimd.dma_start(out=gam_row, in_=d["gamma"][:, :])
    bet_row = const.tile([1, C], F32)
    nc.gpsimd.dma_start(out=bet_row, in_=d["beta"][:, :])

    # per-partition bias columns [128,1] x 2 channel-halves
    bias_cols = {}
    for nm in ("bq", "bk", "bv", "bo"):
        cols = []
        for ch in range(2):
            t = const.tile([P, 1], F32, name=f"{nm}_{ch}")
            nc.gpsimd.dma_start(out=t, in_=d[nm][ch * P:(ch + 1) * P, :])
            cols.append(t)
        bias_cols[nm] = cols

    # ---- GroupNorm statistics (bn_stats already issued in the DMA loop) ----
    # Per channel-half: (mean, var) from bn_aggr, then ex2 = mean^2 + var as a
    # column op so the transpose directly yields (mean | ex2) rows.
    mv = []
    for ch in range(2):
        m = small.tile([P, 2], F32, name=f"mv_{ch}")
        nc.vector.bn_aggr(out=m, in_=bn_st[ch])
        me = small.tile([P, 2], F32, name=f"me_{ch}")
        nc.vector.tensor_copy(out=me[:, 0:1], in_=m[:, 0:1])
        nc.vector.scalar_tensor_tensor(
            out=me[:, 1:2], in0=m[:, 0:1], scalar=m[:, 0:1], in1=m[:, 1:2],
            op0=Alu.mult, op1=Alu.add,
        )
        mv.append(me)

    # transpose per-channel mean and ex2 into one row [1, 512]:
    # [mean_c0 | mean_c1 | ex2_c0 | ex2_c1]
    tp = ps.tile([1, 4 * P], F32, name="tp_stat", tag="po", bufs=1)
    for ch in range(2):
        for k in range(2):
            nc.tensor.transpose(
                tp[:, (2 * k + ch) * P:(2 * k + ch + 1) * P], mv[ch][:, k:k + 1],
                ident,
            )
    mrows = small.tile([1, 4 * P], F32)
    nc.vector.tensor_copy(out=mrows, in_=tp)
    mean_row = mrows[:, 0:C]
    ex2_row = mrows[:, C:2 * C]

    warm(8)  # keep the PE activity monitor warm through the stats dance

    # group sums (over the 8 channels of each group)
    m_s = small.tile([1, G], F32)
    nc.vector.tensor_reduce(
        out=m_s, in_=mean_row.rearrange("o (g j) -> o g j", j=CPG), axis=Axis.X,
        op=Alu.add,
    )
    e_s = small.tile([1, G], F32)
    nc.vector.tensor_reduce(
        out=e_s, in_=ex2_row.rearrange("o (g j) -> o g j", j=CPG), axis=Axis.X,
        op=Alu.add,
    )
    # dependent mini-warm: fires exactly when the reduce lands, keeping the
    # activity monitor fed through the middle of the dance
    nc.tensor.matmul(wu_ps[0:1, 0:G], lhsT=one11, rhs=m_s, start=True, stop=True)
    # var_g = e_s/8 - (m_s/8)^2
    mm_g = small.tile([1, G], F32)
    nc.vector.tensor_mul(mm_g, m_s, m_s)
    mm_g2 = small.tile([1, G], F32)
    nc.vector.tensor_scalar_mul(mm_g2, mm_g, 1.0 / (CPG * CPG))
    var_g = small.tile([1, G], F32)
    nc.vector.scalar_tensor_tensor(
        out=var_g, in0=e_s, scalar=1.0 / CPG, in1=mm_g2, op0=Alu.mult,
        op1=Alu.subtract,
    )
    # rstd_g = 1/sqrt(var_g + eps). The sqrt is issued BEFORE the w_bf casts
    # so the dance's one ACT op isn't queued behind four big weight casts.
    sq_g = small.tile([1, G], F32)
    nc.scalar.activation(sq_g, var_g, Act.Sqrt, bias=eps11, scale=1.0)
    rstd_g = small.tile([1, G], F32)
    nc.vector.reciprocal(rstd_g, sq_g)
    nc.tensor.matmul(wu_ps[0:1, 0:G], lhsT=one11, rhs=rstd_g, start=True, stop=True)

    # ---- weights: cast to bf16 [128(ci), 2(ci_half), 256(co)]; wot also to
    # fp8 x8 for the DoubleRow out-projection ----
    w_bf = {}
    for wi, nm in enumerate(("wqt", "wkt", "wvt", "wot")):
        wb = const.tile([P, 2, C], BF16, name=f"{nm}_bf")
        for ch in range(2):
            nc.scalar.copy(wb[:, ch, :], wall_sb[:, wi, ch, :])
        w_bf[nm] = wb
    wot8 = const.tile([P, 2, C], FP8)
    for ch in range(2):
        nc.scalar.mul(wot8[:, ch, :], wall_sb[:, 3, ch, :], 8.0)

    # broadcast group values to channels: [1,32] -> [1,256] (repeat 8) in one
    # DVE copy via a step-0 read AP
    def grp_bcast(src, name):
        dst = small.tile([1, C], F32, name=name)
        src_ap = src[:, :]
        rep = bass.AP(
            tensor=src_ap.tensor, offset=src_ap.offset,
            ap=[src_ap.ap[0], src_ap.ap[1], [0, CPG]],
        )
        nc.vector.tensor_copy(out=dst.rearrange("o (g j) -> o g j", j=CPG), in_=rep)
        return dst

    rstd_c = grp_bcast(rstd_g, "rstd_c")
    msum_c = grp_bcast(m_s, "msum_c")

    # a = gamma * rstd ; b = beta - (m_s/8) * a    (rows [1,256])
    a_row = small.tile([1, C], F32)
    nc.vector.tensor_mul(a_row, gam_row, rstd_c)
    ma_row = small.tile([1, C], F32)
    nc.vector.scalar_tensor_tensor(
        out=ma_row, in0=msum_c, scalar=1.0 / CPG, in1=a_row, op0=Alu.mult,
        op1=Alu.mult,
    )
    b_row = small.tile([1, C], F32)
    nc.vector.tensor_sub(b_row, bet_row, ma_row)
    nc.tensor.matmul(wu_ps[0:1, 0:C], lhsT=one11, rhs=b_row, start=True, stop=True)

    # transpose a/b rows back to per-partition columns [128,1] per ch-half;
    # the psum tiles use the (idle during the dance) "sps" ring so the four
    # transposes pipeline instead of serializing on one bank
    ab_cols = {"a": [], "b": []}
    for nm, row in (("a", a_row), ("b", b_row)):
        for ch in range(2):
            tp = ps.tile([P, 2, QB], F32, name="tp_ab", tag="sps")[:, 0, 0:1]
            nc.tensor.matmul(
                tp, lhsT=row[:, ch * P:(ch + 1) * P], rhs=one11, start=True,
                stop=True,
            )
            col = small.tile([P, 1], F32, name=f"{nm}_col_{ch}")
            nc.vector.tensor_copy(out=col, in_=tp)
            ab_cols[nm].append(col)

    # ---- fold the norm affine into the projections ----
    # Q = (wq diag(a)) x_bf + (wq b + bq), same for K; V likewise with its
    # constant (wv b + bv) folded through PV/denom into bo_eff.
    b_bf = []
    for ci in range(2):
        t = small.tile([P, 1], BF16, name=f"b_bf_{ci}")
        nc.vector.tensor_copy(out=t, in_=ab_cols["b"][ci])
        b_bf.append(t)

    def matvec_bias(wname, rhs_cols, bias_add, out_dt, out_name):
        outs = []
        for co in range(2):
            pe = ps.tile([P, 2, QB], F32, name="pe_mv", tag="sps")[:, 0, 0:1]
            for ci in range(2):
                nc.tensor.matmul(
                    pe, lhsT=w_bf[wname][:, ci, co * P:(co + 1) * P],
                    rhs=rhs_cols[ci], start=(ci == 0), stop=(ci == 1),
                )
            t = small.tile([P, 1], out_dt, name=f"{out_name}_{co}")
            nc.scalar.activation(
                t, pe, Act.Identity, bias=bias_add[co], scale=1.0
            )
            outs.append(t)
        return outs

    be_q = matvec_bias("wqt", b_bf, bias_cols["bq"], F32, "be_q")
    be_k = matvec_bias("wkt", b_bf, bias_cols["bk"], F32, "be_k")
    vbv_bf = matvec_bias("wvt", b_bf, bias_cols["bv"], BF16, "vbv")
    bo_eff = matvec_bias("wot", vbv_bf, bias_cols["bo"], F32, "bo_eff")
    warm(8)  # cover the be/w8 latency gap before the projections

    # scale wq/wk/wv rows by 8*a (per input channel) into fp8 tiles for the
    # DoubleRow projection matmuls; the 8x (for fp8 dynamic range on the
    # small weight values) is undone by the 1/8 in the PSUM->fp8 casts.
    # Split across DVE and ACT (per-partition AP scale) so wqt is ready
    # ~800ns after the a8 columns and the Q projection starts immediately.
    a8_cols = []
    for ci in range(2):
        t = small.tile([P, 1], F32, name=f"a8_{ci}")
        nc.vector.tensor_scalar_mul(t, ab_cols["a"][ci], 8.0)
        a8_cols.append(t)
    w8 = {}
    for wname in ("wqt", "wkt", "wvt"):
        ws = const.tile([P, 2, C], FP8, name=f"{wname}_8")
        nc.vector.tensor_scalar_mul(ws[:, 0, :], w_bf[wname][:, 0, :], a8_cols[0])
        nc.scalar.activation(
            ws[:, 1, :], w_bf[wname][:, 1, :], Act.Copy, bias=0.0,
            scale=a8_cols[1],
        )
        w8[wname] = ws

    # ---- projections: fp8 DoubleRow matmuls (contraction 256 = 2 ci halves
    # per instruction) over host-supplied x8, PSUM casts apply the 1/8 that
    # undoes the 8x in w8. Q/K in nb-PAIR psum tiles (4KB) so one [P, 1024]
    # cast amortizes ACT/DVE per-op overhead. ----
    k_sb = const.tile([P, 2, N], FP8)
    q_sb = const.tile([P, 2, NQ], FP8)

    def proj_pair(wname, dst, be, nbp, co):
        pp = ps.tile([P, 2, QB], F32, name="pp", tag="sps")
        for j in range(2):
            nb = 2 * nbp + j
            nc.tensor.matmul(
                pp[:, j, :], lhsT=w8[wname][:, :, co * P:(co + 1) * P],
                rhs=x8_sb[:, :, nb * QB:(nb + 1) * QB],
                start=True, stop=True, perf_mode=DR,
            )
        # tiny warm matmul: keeps the HAM activity monitor fed while the PE
        # waits on the cast engines (the phase is cast-throughput-bound)
        warm(1, cols=P)
        dcols = dst[:, co, nbp * 2 * QB:(nbp + 1) * 2 * QB]
        if co == 0:
            nc.scalar.activation(
                dcols, pp.rearrange("p j q -> p (j q)"), Act.Identity,
                bias=be[co], scale=0.125,
            )
        else:
            nc.vector.tensor_scalar(
                out=dcols, in0=pp.rearrange("p j q -> p (j q)"),
                scalar1=0.125, scalar2=be[co], op0=Alu.mult, op1=Alu.add,
            )

    for co in range(2):
        proj_pair("wqt", q_sb, be_q, 0, co)
    for nbp in range(4):
        for co in range(2):
            proj_pair("wkt", k_sb, be_k, nbp, co)
    for co in range(2):
        proj_pair("wqt", q_sb, be_q, 1, co)

    # V [N, C] fp8 (bias folded into bo_eff) in 4-chunk psum tiles:
    # psum[:, n2*C:+C] = sum_ci x8_chunk.T @ w8v
    v_sb = const.tile([P, NKT, C], FP8)
    v_flat = v_sb.rearrange("p k c -> p (k c)")
    for nt in range(0, NKT, 4):
        pv = ps.tile([P, 4 * C], F32, name="pv", tag="sps")
        for n2 in range(4):
            nc.tensor.matmul(
                pv[:, n2 * C:(n2 + 1) * C],
                lhsT=x8_sb[:, :, (nt + n2) * P:(nt + n2 + 1) * P],
                rhs=w8["wvt"][:, :, :],
                start=True, stop=True, perf_mode=DR,
            )
        warm(1, cols=P)
        if (nt // 4) % 2 == 0:
            nc.scalar.mul(v_flat[:, nt * C:(nt + 4) * C], pv, 0.125)
        else:
            nc.vector.tensor_scalar(
                out=v_flat[:, nt * C:(nt + 4) * C], in0=pv,
                scalar1=0.125, scalar2=None, op0=Alu.mult,
            )

    # ---- attention, per query block; key tiles processed in PAIRS with fp8
    # DoubleRow matmuls (contraction 256 per instruction). The softmax
    # denominator accumulates on the PE as a ones-vector DoubleRow matmul
    # into dps [1, 512]. The division is commuted through the out-projection:
    # out = (wo @ (P.V)) * (1/denom) + bo_eff + x.
    def epilogue(qb, dps, aps, last=False):
        # casts first (both DVE; scale 1/8 for the fp8 out-projection): they
        # release the PV accumulator banks immediately
        at8 = work.tile([P, 2, QB], FP8, name="at8", tag="at8", bufs=2)
        for ci in range(2):
            nc.vector.tensor_scalar(
                out=at8[:, ci, :], in0=aps[ci],
                scalar1=0.125, scalar2=None, op0=Alu.mult,
            )
        den_r = work.tile([1, QB], F32, name="den_r", tag="den_r")
        nc.vector.reciprocal_approx_fast(out=den_r, in_=dps)
        den_b = work.tile([P, QB], F32, name="den_b", tag="den_b", bufs=2)
        nc.gpsimd.partition_broadcast(den_b, den_r)
        for co in range(2):
            po = ps.tile([P, QB], F32, name="po", tag="po", bufs=1)
            nc.tensor.matmul(
                po, lhsT=wot8[:, :, co * P:(co + 1) * P],
                rhs=at8[:, :, :], start=True, stop=True, perf_mode=DR,
            )
            t1 = work.tile([P, QB], F32, name="t1", tag="t1")
            nc.vector.tensor_mul(t1, po, den_b)
            res = work.tile([P, QB], BF16, name="res", tag="res", bufs=4)
            nc.vector.scalar_tensor_tensor(
                out=res, in0=t1, scalar=bo_eff[co],
                in1=x_sb[:, co, qb * QB:(qb + 1) * QB], op0=Alu.add, op1=Alu.add,
            )
            eng = nc.sync if co == 0 else nc.scalar
            eng.dma_start(
                out=out_d[co * P:(co + 1) * P, qb * QB:(qb + 1) * QB], in_=res
            )

    pending = None
    for qb in range(NQB):
        p_sb = pblk.tile([P, NKT, QB], FP8, name="p_sb")
        dps = ps_d.tile([1, QB], F32, name="dps", tag="dps")
        aps = [
            ps_acc.tile([P, QB], F32, name="aps", tag="acc") for _ in range(2)
        ]
        for kp in range(NKP + 2):
            if kp == 1 and pending is not None:
                # previous qb's epilogue goes FIRST so its at_sb casts
                # precede this qb's exps in the ACT/DVE program order
                epilogue(*pending)
                pending = None
            if kp < NKP:
                sps2 = ps.tile([P, 2, QB], F32, name="sps2", tag="sps")
                for j in range(2):
                    kt = 2 * kp + j
                    nc.tensor.matmul(
                        sps2[:, j, :],
                        lhsT=k_sb[:, :, kt * P:(kt + 1) * P],
                        rhs=q_sb[:, :, qb * QB:(qb + 1) * QB],
                        start=True, stop=True, perf_mode=DR,
                    )
                nc.scalar.activation(
                    p_sb[:, 2 * kp:2 * kp + 2, :].rearrange("p k q -> p (k q)"),
                    sps2.rearrange("p k q -> p (k q)"),
                    Act.Exp, scale=SCALE, bias=shift_col,
                )
            if kp >= 2:
                pk = kp - 2
                nc.tensor.matmul(
                    dps, lhsT=ones8[:, :, 0:1],
                    rhs=p_sb[:, 2 * pk:2 * pk + 2, :],
                    start=(pk == 0), stop=(pk == NKP - 1),
                    perf_mode=DR, skip_group_check=True,
                )
                for ch in range(2):
                    nc.tensor.matmul(
                        aps[ch],
                        lhsT=v_sb[:, 2 * pk:2 * pk + 2, ch * P:(ch + 1) * P],
                        rhs=p_sb[:, 2 * pk:2 * pk + 2, :],
                        start=(pk == 0), stop=(pk == NKP - 1),
                        perf_mode=DR, skip_group_check=True,
                    )
        pending = (qb, dps, aps)
    epilogue(*pending, last=True)

    for pool in (ps_d, ps_acc, ps, work, pblk, small, const):
        pool.release()


def build_program():
    global _NC
    if _NC is not None:
        return _NC
    nc = bacc.Bacc("TRN2", target_bir_lowering=False, debug=False,
                   num_devices=NCORES)
    d = {
        "x": nc.dram_tensor("x", [C, N], BF16, kind="ExternalInput"),
        "x8": nc.dram_tensor("x8", [C, N], FP8, kind="ExternalInput"),
        "wall": nc.dram_tensor("wall", [P, 4 * 2 * C], F32, kind="ExternalInput"),
        "bq": nc.dram_tensor("bq", [C, 1], F32, kind="ExternalInput"),
        "bk": nc.dram_tensor("bk", [C, 1], F32, kind="ExternalInput"),
        "bv": nc.dram_tensor("bv", [C, 1], F32, kind="ExternalInput"),
        "bo": nc.dram_tensor("bo", [C, 1], F32, kind="ExternalInput"),
        "gamma": nc.dram_tensor("gamma", [1, C], F32, kind="ExternalInput"),
        "beta": nc.dram_tensor("beta", [1, C], F32, kind="ExternalInput"),
        "out": nc.dram_tensor("out", [C, NQ], BF16, kind="ExternalOutput"),
    }
    with tile.TileContext(nc) as tc:
        _body(tc, d)
    nc.compile()
    _NC = nc
    return nc


def make_in_maps(x, gamma, beta, wq, bq, wk, bk, wv, bv, wo, bo):
    f32c = lambda a: np.ascontiguousarray(np.asarray(a, dtype=np.float32))
    x = f32c(x)
    # wall[p, k, h, co] = w_k^T[h*128+p, co]  (k in {q,k,v,o})
    wall = np.stack([
        np.asarray(w, np.float32).T for w in (wq, wk, wv, wo)
    ]).reshape(4, 2, P, C).transpose(2, 0, 1, 3).reshape(P, 4 * 2 * C)
    base = {
        "wall": f32c(wall),
        "bq": f32c(bq).reshape(C, 1),
        "bk": f32c(bk).reshape(C, 1),
        "bv": f32c(bv).reshape(C, 1),
        "bo": f32c(bo).reshape(C, 1),
        "gamma": f32c(gamma).reshape(1, C),
        "beta": f32c(beta).reshape(1, C),
    }
    import ml_dtypes

    in_maps = []
    for core in range(NCORES):
        b, h = divmod(core, 2)
        xb = x[b].reshape(C, N)
        if h:
            xb = np.concatenate([xb[:, NQ:], xb[:, :NQ]], axis=1)
        in_maps.append({
            **base,
            "x": np.ascontiguousarray(xb.astype(ml_dtypes.bfloat16)),
            "x8": np.ascontiguousarray(xb.astype(ml_dtypes.float8_e4m3)),
        })
    return in_maps


def kernel(x, gamma, beta, wq, bq, wk, bk, wv, bv, wo, bo):
    global LAST_RESULTS
    from concourse.bass_utils import run_bass_kernel_spmd

    nc = build_program()
    in_maps = make_in_maps(x, gamma, beta, wq, bq, wk, bk, wv, bv, wo, bo)
    res = run_bass_kernel_spmd(nc, in_maps, core_ids=list(range(NCORES)))
    LAST_RESULTS = res
    out = np.empty((B, C, N), np.float32)
    for core in range(NCORES):
        b, h = divmod(core, 2)
        out[b][:, h * NQ:(h + 1) * NQ] = np.asarray(
            res.results[core]["out"], dtype=np.float32
        )
    return out.reshape(B, C, H, W)
